# revision 1
# baseline (speedup 1.0000x reference)
"""Multi-head attention (S=2048, B=2, D=1024, H=16) on 8 trn2 NeuronCores.

Sharding: 2 heads per core (head parallelism). Each core computes Q/K/V
projections for its 128 output features, attention for its 4 (batch,
head) pairs, and a partial output projection; the host sums the 8
partial outputs.

v2 restructure vs the 261us baseline (which idled the Scalar/exp engine
107us): the exp stream is the kernel's hard floor (134M exps / core /
153.6G/s ~= 110us + per-instruction overhead = 147us), so everything is
scheduled around keeping it continuous:
 - batch-major token layout ([D, B*S]) kills the strided deinterleave
   copies and lets attention chunks depend on one batch's K/V only.
 - b-outer chunk order (b0 qc0-3 then b1 qc0-3) with a hand-ordered DMA
   priority stream: k(b0) + q(b0,0) first, so exp starts ~10us in
   instead of 30us, and b1's inputs stream under b0's attention.
 - per-slot emission order [exp(g) | scores(g+2) | folds | av(g)]
   keeps the scores that gate the NEXT exp ahead of all other PE work
   in the engine FIFO (scores lead the exp stream by 2 key-tiles,
   crossing chunk boundaries), so av/proj/oproj stalls never block it.
 - q/k/v projections for later chunks, v transposes, and the previous
   chunks' output projections are folded into explicit slots of the
   attention stream, each sized under the ~290ns/slot PE slack.
 - kT/qT/xT/wo are fp16 (half DMA + SBUF, FWL weight loads); scores /
   exp / attention accumulate stay fp32/f32r.  Partial outputs ship
   fp16 (halves the 16MB output traffic; host sums in fp32).
"""
import sys
sys.path.insert(0, '/opt/trn_rl_repo')
import functools
import os

import numpy as np

import concourse.bacc as bacc
import concourse.mybir as mybir
import concourse.tile as tile
from concourse.bass_utils import run_bass_kernel_spmd
from concourse.masks import make_identity

F32 = mybir.dt.float32
F32R = mybir.dt.float32r
F16 = mybir.dt.float16
BF16 = mybir.dt.bfloat16
AFT = mybir.ActivationFunctionType
MUL = mybir.AluOpType.mult

S, B, D, H = 2048, 2, 1024, 16
T = S * B               # 4096 tokens
DK = D // H             # 64
NC = 8                  # cores
FPC = D // NC           # 128 features per core (2 heads)
QC = 512                # q-chunk size
JT = S // 128           # 16 key tiles per batch
DT = D // 128           # 8 contraction tiles for projections
NCH = 8                 # chunks, b-outer: c -> b=c//4, qc=c%4
NSLOT = NCH * JT        # 128 global (chunk, key-tile) slots

EJ_DT = {"f32r": F32R, "bf16": BF16}[os.environ.get("EJ_DT", "f32r")]
OUT_DT = {"fp16": F16, "fp32": F32}[os.environ.get("OUT_DT", "fp16")]


def build_nc():
    nc = bacc.Bacc(None, target_bir_lowering=False)

    xq = nc.dram_tensor("xq", [D, T], F16, kind="ExternalInput")
    xk = nc.dram_tensor("xk", [D, T], F16, kind="ExternalInput")
    xv = nc.dram_tensor("xv", [D, T], F16, kind="ExternalInput")
    wq = nc.dram_tensor("wq", [D, FPC], F16, kind="ExternalInput")
    wk = nc.dram_tensor("wk", [D, FPC], F16, kind="ExternalInput")
    wv = nc.dram_tensor("wv", [D, FPC], F16, kind="ExternalInput")
    wo = nc.dram_tensor("wo", [FPC, D], F16, kind="ExternalInput")
    out = nc.dram_tensor("out", [T, D], OUT_DT, kind="ExternalOutput")
    DBG = os.environ.get("DEBUG_DUMP", "0") == "1"
    if DBG:
        dbg_q = nc.dram_tensor("dbg_q", [128, S], F16, kind="ExternalOutput")
        dbg_k = nc.dram_tensor("dbg_k", [128, S], F16, kind="ExternalOutput")
        dbg_vb = nc.dram_tensor("dbg_vb", [128, JT * 130], F32, kind="ExternalOutput")
        dbg_den = nc.dram_tensor("dbg_den", [2, QC], F32, kind="ExternalOutput")
        dbg_ej = nc.dram_tensor("dbg_ej", [128, 2 * QC], F32, kind="ExternalOutput")
        dbg_ej2 = nc.dram_tensor("dbg_ej2", [128, 2 * QC], F32, kind="ExternalOutput")
        dbg_p0 = nc.dram_tensor("dbg_p0", [65, QC], F32, kind="ExternalOutput")
        dbg_p1 = nc.dram_tensor("dbg_p1", [65, QC], F32, kind="ExternalOutput")
        dbg_snap = nc.dram_tensor("dbg_snap", [3, 65, QC], F32, kind="ExternalOutput")
    xsrc = {"q": xq, "k": xk, "v": xv}

    with tile.TileContext(nc) as tc:
        with (
            tc.tile_pool(name="wpool", bufs=1) as wpool,
            tc.tile_pool(name="proj", bufs=1) as projpool,
            tc.tile_pool(name="vtmp", bufs=2) as vtpool,
            tc.tile_pool(name="xdma", bufs=6) as xpool,
            tc.tile_pool(name="ej", bufs=16) as epool,
            tc.tile_pool(name="norm", bufs=2) as npool,
            tc.tile_pool(name="osb", bufs=3) as opool,
            tc.tile_pool(name="psS", bufs=2, space="PSUM") as psS,
            tc.tile_pool(name="psA", bufs=2, space="PSUM") as psA,
            tc.tile_pool(name="psM", bufs=2, space="PSUM") as psM,
        ):
            # ---- weights / constants ----
            w_t = {}
            for name, wd in (("k", wk), ("q", wq), ("v", wv)):
                w_t[name] = wpool.tile([128, DT, FPC], F16, name=f"w_{name}")
                nc.sync.dma_start(w_t[name][:], wd.rearrange("(t p) m -> p t m", p=128))
            ident = wpool.tile([128, 128], F32, name="ident")
            make_identity(nc, ident[:])
            wo_t = wpool.tile([128, D], F16, name="wo_t")

            # ---- persistent activations ----
            kT = [projpool.tile([128, S], F16, name=f"kT{b}") for b in range(B)]
            qT = [projpool.tile([128, S], F16, name=f"qT{b}") for b in range(B)]
            v_b = [projpool.tile([128, JT, 130], F32R, name=f"v_b{b}") for b in range(B)]
            xT = projpool.tile([128, T], F16, name="xT")
            # ones columns of v_b (denominator trick) are static
            for b in range(B):
                for jt in range(JT):
                    nc.vector.memset(v_b[b][:, jt, 64:65].bitcast(F32), 1.0)
                    nc.vector.memset(v_b[b][:, jt, 129:130].bitcast(F32), 1.0)

            # ---- input DMA stream (emission order == transfer order) ----
            xt_tiles = {}

            def emit_xdma(p, b, hf):
                t = xpool.tile([128, DT, 512], F16, name="xt", tag="xt")
                tok0 = b * S + hf * 512
                nc.sync.dma_start(
                    t[:],
                    xsrc[p].rearrange("(a p) m -> p a m", p=128)[:, :, tok0:tok0 + 512])
                xt_tiles[(p, b, hf)] = t

            dma_order = [
                ("k", 0, 0), ("q", 0, 0), ("v", 0, 0), ("v", 0, 1), ("v", 0, 2),
                ("v", 0, 3), ("k", 0, 1), ("k", 0, 2), ("k", 0, 3), ("q", 0, 1),
                "WO",
                ("q", 0, 2), ("k", 1, 0), ("k", 1, 1), ("k", 1, 2),
                ("k", 1, 3), ("q", 0, 3), ("v", 1, 0), ("v", 1, 1), ("v", 1, 2),
                ("v", 1, 3), ("q", 1, 0), ("q", 1, 1), ("q", 1, 2), ("q", 1, 3),
            ]
            for item in dma_order:
                if item == "WO":
                    nc.sync.dma_start(wo_t[:], wo[:, :])
                else:
                    emit_xdma(*item)

            # ---- building blocks ----
            def proj_mms(p, b, hf, dest):
                """8 accumulating matmuls + 1 evac copy for 512 tokens."""
                xt = xt_tiles.pop((p, b, hf))
                ps = psM.tile([128, 512], F32, name="psproj", tag="m")
                for dt in range(DT):
                    nc.tensor.matmul(ps[:], w_t[p][:, dt, :], xt[:, dt, :],
                                     start=(dt == 0), stop=(dt == DT - 1))
                nc.vector.tensor_copy(dest, ps[:])

            vstore = {}

            def v_proj(b, hf):
                vt = vtpool.tile([128, 512], F32R, name="vt", tag="vt")
                proj_mms("v", b, hf, vt[:])
                vstore[(b, hf)] = vt

            def v_tp(b, hf, jj):
                """transpose 2 of the 4 key-tiles of v half hf into v_b."""
                vt = vstore[(b, hf)]
                for u in range(2):
                    q = 2 * jj + u
                    jt = 4 * hf + q
                    tp = psM.tile([128, 128], F32, name="tp", tag="m")
                    nc.tensor.transpose(
                        tp[:], vt[:, q * 128:(q + 1) * 128].bitcast(F32), ident[:])
                    nc.vector.tensor_copy(v_b[b][:, jt, 0:64], tp[:, 0:64])
                    nc.vector.tensor_copy(v_b[b][:, jt, 65:129], tp[:, 64:128])

            sj_t = {}
            ej_t = {}

            def emit_scores(g):
                c, j = divmod(g, JT)
                b, qc = divmod(c, 4)
                sj = psS.tile([128, 2, QC], F32, name="sj", tag="sj")
                for h in range(2):
                    nc.tensor.matmul(
                        sj[:, h, :], kT[b][h * 64:(h + 1) * 64, j * 128:(j + 1) * 128],
                        qT[b][h * 64:(h + 1) * 64, qc * QC:(qc + 1) * QC],
                        start=True, stop=True)
                sj_t[g] = sj

            def emit_exp(g):
                ej = epool.tile([128, 2, QC], EJ_DT, name="ej", tag="ej")
                nc.scalar.activation(ej[:], sj_t.pop(g)[:], AFT.Exp)
                if DBG and g in (0, 15):
                    ejd = npool.tile([128, 2 * QC], F32, name="ejd", tag="ejd")
                    nc.vector.tensor_copy(ejd[:], ej[:].rearrange("p a m -> p (a m)"))
                    nc.sync.dma_start((dbg_ej if g == 0 else dbg_ej2)[:, :], ejd[:])
                ej_t[g] = ej

            def emit_av(g, pacc):
                c, j = divmod(g, JT)
                b = c // 4
                ej = ej_t.pop(g)
                for h in range(2):
                    nc.tensor.matmul(
                        pacc[h][0:65, :], v_b[b][:, j, h * 65:h * 65 + 65],
                        ej[:, h, :], start=(j == 0), stop=(j == JT - 1))

            def emit_normalize(c, pacc):
                if DBG and c == 0:
                    for h, dst in ((0, dbg_p0), (1, dbg_p1)):
                        pd = npool.tile([65, QC], F32, name="pd", tag="pd")
                        nc.vector.tensor_copy(pd[:], pacc[h][0:65, :])
                        nc.sync.dma_start(dst[:, :], pd[:])
                for h in range(2):
                    dsb = npool.tile([1, QC], F32, name="dsb", tag="dsb")
                    nc.vector.tensor_copy(dsb[:], pacc[h][64:65, :])
                    if DBG and c == 0:
                        nc.sync.dma_start(dbg_den[h:h + 1, :], dsb[:])
                    rd = npool.tile([1, QC], F32, name="rd", tag="rd")
                    nc.vector.reciprocal_approx_fast(out=rd[:], in_=dsb[:])
                    bc = npool.tile([64, QC], F32, name="bc", tag="bc")
                    nc.gpsimd.partition_broadcast(bc[:], rd[:])
                    nc.vector.tensor_tensor(
                        out=xT[h * 64:(h + 1) * 64, c * QC:(c + 1) * QC],
                        in0=pacc[h][0:64, :], in1=bc[:], op=MUL)

            def emit_oproj(tt, use_act=False):
                osb = opool.tile([128, D], OUT_DT, name="osb", tag="osb")
                for ec in range(2):
                    po = psM.tile([128, 512], F32, name="po", tag="m")
                    nc.tensor.matmul(po[:], xT[:, tt * 128:(tt + 1) * 128],
                                     wo_t[:, ec * 512:(ec + 1) * 512],
                                     start=True, stop=True)
                    if use_act and ec == 0:
                        nc.scalar.copy(osb[:, 0:512], po[:])
                    else:
                        nc.vector.tensor_copy(osb[:, ec * 512:(ec + 1) * 512], po[:])
                nc.gpsimd.dma_start(out[tt * 128:(tt + 1) * 128, :], osb[:])

            # ---- fold schedule: slot -> thunks (sized ~<=1.7us PE each) ----
            def k_item(b, hf):
                return lambda: proj_mms("k", b, hf, kT[b][:, hf * 512:(hf + 1) * 512])

            def q_item(c):
                b, qc = divmod(c, 4)
                return lambda: proj_mms("q", b, qc, qT[b][:, qc * 512:(qc + 1) * 512])

            def vp_item(b, hf):
                return lambda: v_proj(b, hf)

            def vt_item(b, hf, jj):
                return lambda: v_tp(b, hf, jj)

            def op_item(tt):
                return lambda: emit_oproj(tt)

            FOLD = {}

            def put(g, th):
                FOLD.setdefault(g, []).append(th)

            # b0 k halves 1-3 fold early in chunk 0 (v(b0) is all done in
            # the prologue so chunk 0's avs run at their natural slots)
            put(0, k_item(0, 1)); put(4, k_item(0, 2)); put(8, k_item(0, 3))
            # q projections: chunk c's q one chunk ahead (c0's q in prologue)
            put(13, q_item(1)); put(29, q_item(2)); put(45, q_item(3))
            put(60, q_item(4)); put(77, q_item(5)); put(93, q_item(6)); put(109, q_item(7))
            # b1 k/v prep under b0's chunks 2-3
            put(34, k_item(1, 0)); put(37, k_item(1, 1))
            put(40, k_item(1, 2)); put(43, k_item(1, 3))
            put(50, vp_item(1, 0)); put(51, vt_item(1, 0, 0)); put(52, vt_item(1, 0, 1))
            put(53, vp_item(1, 1)); put(54, vt_item(1, 1, 0)); put(55, vt_item(1, 1, 1))
            put(56, vp_item(1, 2)); put(57, vt_item(1, 2, 0)); put(58, vt_item(1, 2, 1))
            put(59, vp_item(1, 3)); put(61, vt_item(1, 3, 0)); put(62, vt_item(1, 3, 1))
            # output projection tiles: chunk c's 4 tiles folded 1-2 chunks later
            op_slots = {0: (25, 27, 30, 31), 1: (38, 41, 44, 46),
                        2: (70, 72, 74, 76), 3: (82, 84, 86, 88),
                        4: (98, 100, 102, 104), 5: (114, 116, 118, 120),
                        6: (121, 123, 125, 127)}
            for c, slots in op_slots.items():
                for i, g in enumerate(slots):
                    put(g, op_item(4 * c + i))

            # ---- prologue compute ----
            proj_mms("k", 0, 0, kT[0][:, 0:512])
            proj_mms("q", 0, 0, qT[0][:, 0:512])
            for hf in range(4):
                v_proj(0, hf)
                v_tp(0, hf, 0)
                v_tp(0, hf, 1)
            emit_scores(0)
            emit_scores(1)

            # ---- av emission slots: an av can only be emitted once the
            # v_tp that writes its v_b key-tile has been emitted (chunk 0
            # streams under the input DMA, so its avs defer), and a chunk's
            # avs must all precede the next chunk's (pacc bank rotation).
            AV_SLOT = {}
            for g in range(NSLOT):
                AV_SLOT.setdefault(g, []).append(g)

            # ---- the stream ----
            paccs = {}
            for g in range(NSLOT):
                c, j = divmod(g, JT)
                if j == 0:
                    paccs[c] = [psA.tile([128, QC], F32, name=f"pacc{h}", tag="pacc")
                                for h in range(2)]
                emit_exp(g)
                if g + 2 < NSLOT:
                    emit_scores(g + 2)
                for th in FOLD.get(g, ()):
                    th()
                for ga in AV_SLOT.get(g, ()):
                    ca = ga // JT
                    emit_av(ga, paccs[ca])
                    if DBG and ca == 0 and ga in (3, 7, 11):
                        pd = npool.tile([65, QC], F32, name="pd", tag="pd")
                        nc.vector.tensor_copy(pd[:], paccs[0][0][0:65, :])
                        nc.sync.dma_start(dbg_snap[ga // 4, :, :], pd[:])
                    if ga % JT == JT - 1:
                        emit_normalize(ca, paccs.pop(ca))

            # ---- tail: last chunk's output projection ----
            for tt in range(28, 32):
                emit_oproj(tt, use_act=True)
            if DBG:
                nc.sync.dma_start(dbg_q[:, :], qT[0][:])
                nc.sync.dma_start(dbg_k[:, :], kT[0][:])
                nc.sync.dma_start(
                    dbg_vb[:, :], v_b[0][:].bitcast(F32).rearrange("p a m -> p (a m)"))
    nc.finalize()
    return nc


@functools.cache
def _nc_cached():
    return build_nc()


def _prep_in_maps(inputs):
    np16 = np.float16

    def xbm(a):
        # [S,B,D] -> [D, B*S] batch-major tokens, fp16
        a = np.asarray(a, np.float32).transpose(2, 1, 0).reshape(D, T)
        return np.ascontiguousarray(a).astype(np16)

    xq_h = xbm(inputs["query"])
    xk_h = xbm(inputs["key"])
    xv_h = xbm(inputs["value"])
    Wq, Wk, Wv, Wo = (np.asarray(inputs[k], np.float32) for k in ("Wq", "Wk", "Wv", "Wo"))

    in_maps = []
    for c in range(NC):
        sl = slice(c * FPC, (c + 1) * FPC)
        in_maps.append({
            "xq": xq_h, "xk": xk_h, "xv": xv_h,
            "wq": np.ascontiguousarray(Wq[sl, :].T).astype(np16),
            "wk": np.ascontiguousarray(Wk[sl, :].T).astype(np16),
            "wv": np.ascontiguousarray(Wv[sl, :].T).astype(np16),
            "wo": np.ascontiguousarray(Wo[:, sl].T).astype(np16),
        })
    return in_maps


def kernel(query, key, value, Wq, bq, Wk, bk, Wv, bv, Wo, bo):
    in_maps = _prep_in_maps({"query": query, "key": key, "value": value,
                             "Wq": Wq, "Wk": Wk, "Wv": Wv, "Wo": Wo})
    nc = _nc_cached()
    res = run_bass_kernel_spmd(nc, in_maps, core_ids=list(range(NC)))
    acc = np.zeros((T, D), np.float32)
    for r in res.results:
        acc += r["out"].astype(np.float32)
    acc += np.asarray(bo, np.float32)[None, :]
    # batch-major tokens back to [S, B, D]
    out = acc.reshape(B, S, D).transpose(1, 0, 2)
    out = np.ascontiguousarray(out, np.float32)
    for bias in (bq, bk, bv):
        assert float(np.abs(np.asarray(bias)).max()) == 0.0, "nonzero qkv bias unsupported"
    return out



# revision 11
# speedup vs baseline: 1.0020x; 1.0020x over previous
"""Multi-head attention (S=2048, B=2, D=1024, H=16) on 8 trn2 NeuronCores.

Sharding: 2 heads per core (head parallelism). Each core computes Q/K/V
projections for its 128 output features, attention for its 4 (batch,
head) pairs, and a partial output projection; the host sums the 8
partial outputs.

v2 restructure vs the 261us baseline (which idled the Scalar/exp engine
107us): the exp stream is the kernel's hard floor (134M exps / core /
153.6G/s ~= 110us + per-instruction overhead = 147us), so everything is
scheduled around keeping it continuous:
 - batch-major token layout ([D, B*S]) kills the strided deinterleave
   copies and lets attention chunks depend on one batch's K/V only.
 - b-outer chunk order (b0 qc0-3 then b1 qc0-3) with a hand-ordered DMA
   priority stream: k(b0) + q(b0,0) first, so exp starts ~10us in
   instead of 30us, and b1's inputs stream under b0's attention.
 - per-slot emission order [exp(g) | scores(g+2) | folds | av(g)]
   keeps the scores that gate the NEXT exp ahead of all other PE work
   in the engine FIFO (scores lead the exp stream by 2 key-tiles,
   crossing chunk boundaries), so av/proj/oproj stalls never block it.
 - q/k/v projections for later chunks, v transposes, and the previous
   chunks' output projections are folded into explicit slots of the
   attention stream, each sized under the ~290ns/slot PE slack.
 - kT/qT/xT/wo are fp16 (half DMA + SBUF, FWL weight loads); scores /
   exp / attention accumulate stay fp32/f32r.  Partial outputs ship
   fp16 (halves the 16MB output traffic; host sums in fp32).
"""
import sys
sys.path.insert(0, '/opt/trn_rl_repo')
import functools
import os

import numpy as np

import concourse.bacc as bacc
import concourse.mybir as mybir
import concourse.tile as tile
from concourse.bass_utils import run_bass_kernel_spmd
from concourse.masks import make_identity

F32 = mybir.dt.float32
F32R = mybir.dt.float32r
F16 = mybir.dt.float16
BF16 = mybir.dt.bfloat16
AFT = mybir.ActivationFunctionType
MUL = mybir.AluOpType.mult

S, B, D, H = 2048, 2, 1024, 16
T = S * B               # 4096 tokens
DK = D // H             # 64
NC = 8                  # cores
FPC = D // NC           # 128 features per core (2 heads)
QC = 512                # q-chunk size
JT = S // 128           # 16 key tiles per batch
DT = D // 128           # 8 contraction tiles for projections
NCH = 8                 # chunks, b-outer: c -> b=c//4, qc=c%4
NSLOT = NCH * JT        # 128 global (chunk, key-tile) slots

EJ_DT = {"f32r": F32R, "bf16": BF16}[os.environ.get("EJ_DT", "bf16")]
OUT_DT = {"fp16": F16, "fp32": F32}[os.environ.get("OUT_DT", "fp16")]


def build_nc():
    nc = bacc.Bacc(None, target_bir_lowering=False)

    xq = nc.dram_tensor("xq", [D, T], F16, kind="ExternalInput")
    xk = nc.dram_tensor("xk", [D, T], F16, kind="ExternalInput")
    xv = nc.dram_tensor("xv", [D, T], F16, kind="ExternalInput")
    wq = nc.dram_tensor("wq", [128, DT * FPC], F16, kind="ExternalInput")
    wk = nc.dram_tensor("wk", [128, DT * FPC], F16, kind="ExternalInput")
    wv = nc.dram_tensor("wv", [128, DT * FPC], F16, kind="ExternalInput")
    wo = nc.dram_tensor("wo", [FPC, D], F16, kind="ExternalInput")
    out = nc.dram_tensor("out", [T, D], OUT_DT, kind="ExternalOutput")
    DBG = os.environ.get("DEBUG_DUMP", "0") == "1"
    if DBG:
        dbg_q = nc.dram_tensor("dbg_q", [128, S], F16, kind="ExternalOutput")
        dbg_k = nc.dram_tensor("dbg_k", [128, S], F16, kind="ExternalOutput")
        dbg_vb = nc.dram_tensor("dbg_vb", [128, JT * 130], F32, kind="ExternalOutput")
        dbg_den = nc.dram_tensor("dbg_den", [2, QC], F32, kind="ExternalOutput")
        dbg_ej = nc.dram_tensor("dbg_ej", [128, 2 * QC], F32, kind="ExternalOutput")
        dbg_ej2 = nc.dram_tensor("dbg_ej2", [128, 2 * QC], F32, kind="ExternalOutput")
        dbg_p0 = nc.dram_tensor("dbg_p0", [65, QC], F32, kind="ExternalOutput")
        dbg_p1 = nc.dram_tensor("dbg_p1", [65, QC], F32, kind="ExternalOutput")
        dbg_snap = nc.dram_tensor("dbg_snap", [3, 65, QC], F32, kind="ExternalOutput")
    xsrc = {"q": xq, "k": xk, "v": xv}

    with tile.TileContext(nc) as tc:
        with (
            tc.tile_pool(name="wpool", bufs=1) as wpool,
            tc.tile_pool(name="proj", bufs=1) as projpool,
            tc.tile_pool(name="vtmp", bufs=2) as vtpool,
            tc.tile_pool(name="xdma", bufs=6) as xpool,
            tc.tile_pool(name="ej", bufs=16) as epool,
            tc.tile_pool(name="norm", bufs=2) as npool,
            tc.tile_pool(name="osb", bufs=3) as opool,
            tc.tile_pool(name="psS", bufs=2, space="PSUM") as psS,
            tc.tile_pool(name="psA", bufs=2, space="PSUM") as psA,
            tc.tile_pool(name="psM", bufs=2, space="PSUM") as psM,
        ):
            # ---- weights / constants ----
            w_t = {}
            for name, wd in (("k", wk), ("q", wq), ("v", wv)):
                w_t[name] = wpool.tile([128, DT, FPC], F16, name=f"w_{name}")
                # weights pre-packed [128, DT*FPC] host-side: contiguous per
                # partition; issued on the scalar HWDGE ring so the x stream
                # owns the sync ring from t=0.
                nc.scalar.dma_start(w_t[name][:], wd.rearrange("p (t m) -> p t m", t=DT))
            ident = wpool.tile([128, 128], BF16, name="ident")
            make_identity(nc, ident[:])
            wo_t = wpool.tile([128, D], F16, name="wo_t")

            # ---- persistent activations ----
            kT = [projpool.tile([128, S], F16, name=f"kT{b}") for b in range(B)]
            qT = [projpool.tile([128, S], F16, name=f"qT{b}") for b in range(B)]
            v_b = [projpool.tile([128, JT, 130], BF16, name=f"v_b{b}") for b in range(B)]
            xT = projpool.tile([128, T], F16, name="xT")
            # ones columns of v_b (denominator trick) are static
            for b in range(B):
                for jt in range(JT):
                    nc.vector.memset(v_b[b][:, jt, 64:65], 1.0)
                    nc.vector.memset(v_b[b][:, jt, 129:130], 1.0)

            # ---- input DMA stream (emission order == transfer order) ----
            xt_tiles = {}

            def emit_xdma(p, b, hf):
                t = xpool.tile([128, DT, 512], F16, name="xt", tag="xt")
                tok0 = b * S + hf * 512
                nc.sync.dma_start(
                    t[:],
                    xsrc[p].rearrange("(a p) m -> p a m", p=128)[:, :, tok0:tok0 + 512])
                xt_tiles[(p, b, hf)] = t

            dma_order = [
                ("k", 0, 0), ("q", 0, 0), ("v", 0, 0), ("v", 0, 1), ("v", 0, 2),
                ("v", 0, 3), ("k", 0, 1), ("k", 0, 2), ("k", 0, 3), ("q", 0, 1),
                "WO",
                ("q", 0, 2), ("k", 1, 0), ("k", 1, 1), ("k", 1, 2),
                ("k", 1, 3), ("q", 0, 3), ("v", 1, 0), ("v", 1, 1), ("v", 1, 2),
                ("v", 1, 3), ("q", 1, 0), ("q", 1, 1), ("q", 1, 2), ("q", 1, 3),
            ]
            for item in dma_order:
                if item == "WO":
                    nc.sync.dma_start(wo_t[:], wo[:, :])
                else:
                    emit_xdma(*item)

            # ---- building blocks ----
            def proj_mms(p, b, hf, dest):
                """8 accumulating matmuls + 1 evac copy for 512 tokens."""
                xt = xt_tiles.pop((p, b, hf))
                ps = psM.tile([128, 512], F32, name="psproj", tag="m")
                for dt in range(DT):
                    nc.tensor.matmul(ps[:], w_t[p][:, dt, :], xt[:, dt, :],
                                     start=(dt == 0), stop=(dt == DT - 1))
                nc.vector.tensor_copy(dest, ps[:])

            vstore = {}

            def v_proj(b, hf):
                vt = vtpool.tile([128, 512], BF16, name="vt", tag="vt")
                proj_mms("v", b, hf, vt[:])
                vstore[(b, hf)] = vt

            def v_tp(b, hf, jj):
                """transpose 2 of the 4 key-tiles of v half hf into v_b."""
                vt = vstore[(b, hf)]
                for u in range(2):
                    q = 2 * jj + u
                    jt = 4 * hf + q
                    tp = psM.tile([128, 128], BF16, name="tp", tag="m")
                    nc.tensor.transpose(
                        tp[:], vt[:, q * 128:(q + 1) * 128], ident[:])
                    nc.vector.tensor_copy(v_b[b][:, jt, 0:64], tp[:, 0:64])
                    nc.vector.tensor_copy(v_b[b][:, jt, 65:129], tp[:, 64:128])

            sj_t = {}
            ej_t = {}

            def emit_scores(g):
                c, j = divmod(g, JT)
                b, qc = divmod(c, 4)
                sj = psS.tile([128, 2, QC], F32, name="sj", tag="sj")
                for h in range(2):
                    nc.tensor.matmul(
                        sj[:, h, :], kT[b][h * 64:(h + 1) * 64, j * 128:(j + 1) * 128],
                        qT[b][h * 64:(h + 1) * 64, qc * QC:(qc + 1) * QC],
                        start=True, stop=True)
                sj_t[g] = sj

            def emit_exp(g):
                ej = epool.tile([128, 2, QC], EJ_DT, name="ej", tag="ej")
                nc.scalar.activation(ej[:], sj_t.pop(g)[:], AFT.Exp)
                if DBG and g in (0, 15):
                    ejd = npool.tile([128, 2 * QC], F32, name="ejd", tag="ejd")
                    nc.vector.tensor_copy(ejd[:], ej[:].rearrange("p a m -> p (a m)"))
                    nc.sync.dma_start((dbg_ej if g == 0 else dbg_ej2)[:, :], ejd[:])
                ej_t[g] = ej

            def emit_av(g, pacc):
                c, j = divmod(g, JT)
                b = c // 4
                ej = ej_t.pop(g)
                for h in range(2):
                    nc.tensor.matmul(
                        pacc[h][0:65, :], v_b[b][:, j, h * 65:h * 65 + 65],
                        ej[:, h, :], start=(j == 0), stop=(j == JT - 1))

            def emit_normalize(c, pacc):
                if DBG and c == 0:
                    for h, dst in ((0, dbg_p0), (1, dbg_p1)):
                        pd = npool.tile([65, QC], F32, name="pd", tag="pd")
                        nc.vector.tensor_copy(pd[:], pacc[h][0:65, :])
                        nc.sync.dma_start(dst[:, :], pd[:])
                for h in range(2):
                    dsb = npool.tile([1, QC], F32, name="dsb", tag="dsb")
                    nc.vector.tensor_copy(dsb[:], pacc[h][64:65, :])
                    if DBG and c == 0:
                        nc.sync.dma_start(dbg_den[h:h + 1, :], dsb[:])
                    rd = npool.tile([1, QC], F32, name="rd", tag="rd")
                    nc.vector.reciprocal_approx_fast(out=rd[:], in_=dsb[:])
                    bc = npool.tile([64, QC], F32, name="bc", tag="bc")
                    nc.gpsimd.partition_broadcast(bc[:], rd[:])
                    nc.vector.tensor_tensor(
                        out=xT[h * 64:(h + 1) * 64, c * QC:(c + 1) * QC],
                        in0=pacc[h][0:64, :], in1=bc[:], op=MUL)

            def emit_oproj(tt, use_act=False):
                osb = opool.tile([128, D], OUT_DT, name="osb", tag="osb")
                for ec in range(2):
                    po = psM.tile([128, 512], F32, name="po", tag="m")
                    nc.tensor.matmul(po[:], xT[:, tt * 128:(tt + 1) * 128],
                                     wo_t[:, ec * 512:(ec + 1) * 512],
                                     start=True, stop=True)
                    if use_act and ec == 0:
                        nc.scalar.copy(osb[:, 0:512], po[:])
                    else:
                        nc.vector.tensor_copy(osb[:, ec * 512:(ec + 1) * 512], po[:])
                # last two chunks' outputs ride the (empty by then) sync
                # HWDGE ring at full rate; earlier tiles trickle on SWDGE.
                eng = nc.sync if tt >= 24 else nc.gpsimd
                eng.dma_start(out[tt * 128:(tt + 1) * 128, :], osb[:])

            # ---- fold schedule: slot -> thunks (sized ~<=1.7us PE each) ----
            def k_item(b, hf):
                return lambda: proj_mms("k", b, hf, kT[b][:, hf * 512:(hf + 1) * 512])

            def q_item(c):
                b, qc = divmod(c, 4)
                return lambda: proj_mms("q", b, qc, qT[b][:, qc * 512:(qc + 1) * 512])

            def vp_item(b, hf):
                return lambda: v_proj(b, hf)

            def vt_item(b, hf, jj):
                return lambda: v_tp(b, hf, jj)

            def op_item(tt):
                return lambda: emit_oproj(tt)

            FOLD = {}

            def put(g, th):
                FOLD.setdefault(g, []).append(th)

            # b0 k halves 1-3 fold early in chunk 0 (v(b0) is all done in
            # the prologue so chunk 0's avs run at their natural slots)
            put(0, k_item(0, 1)); put(4, k_item(0, 2)); put(8, k_item(0, 3))
            # q projections: chunk c's q one chunk ahead (c0's q in prologue)
            put(13, q_item(1)); put(29, q_item(2)); put(45, q_item(3))
            put(60, q_item(4)); put(77, q_item(5)); put(93, q_item(6)); put(109, q_item(7))
            # b1 k/v prep under b0's chunks 2-3
            put(34, k_item(1, 0)); put(37, k_item(1, 1))
            put(40, k_item(1, 2)); put(43, k_item(1, 3))
            put(50, vp_item(1, 0)); put(51, vt_item(1, 0, 0)); put(52, vt_item(1, 0, 1))
            put(53, vp_item(1, 1)); put(54, vt_item(1, 1, 0)); put(55, vt_item(1, 1, 1))
            put(56, vp_item(1, 2)); put(57, vt_item(1, 2, 0)); put(58, vt_item(1, 2, 1))
            put(59, vp_item(1, 3)); put(61, vt_item(1, 3, 0)); put(62, vt_item(1, 3, 1))
            # output projection tiles: chunk c's 4 tiles folded 1-2 chunks later
            op_slots = {0: (25, 27, 30, 31), 1: (38, 41, 44, 46),
                        2: (70, 72, 74, 76), 3: (82, 84, 86, 88),
                        4: (98, 100, 102, 104), 5: (114, 116, 118, 120),
                        6: (121, 123, 125, 127)}
            for c, slots in op_slots.items():
                for i, g in enumerate(slots):
                    put(g, op_item(4 * c + i))

            # ---- prologue compute ----
            proj_mms("k", 0, 0, kT[0][:, 0:512])
            proj_mms("q", 0, 0, qT[0][:, 0:512])
            for hf in range(4):
                v_proj(0, hf)
                v_tp(0, hf, 0)
                v_tp(0, hf, 1)
            emit_scores(0)
            emit_scores(1)

            # ---- av emission slots: an av can only be emitted once the
            # v_tp that writes its v_b key-tile has been emitted (chunk 0
            # streams under the input DMA, so its avs defer), and a chunk's
            # avs must all precede the next chunk's (pacc bank rotation).
            AV_SLOT = {}
            for g in range(NSLOT):
                AV_SLOT.setdefault(g, []).append(g)

            # ---- the stream ----
            paccs = {}
            for g in range(NSLOT):
                c, j = divmod(g, JT)
                if j == 0:
                    paccs[c] = [psA.tile([128, QC], F32, name=f"pacc{h}", tag="pacc")
                                for h in range(2)]
                emit_exp(g)
                if g + 2 < NSLOT:
                    emit_scores(g + 2)
                for th in FOLD.get(g, ()):
                    th()
                for ga in AV_SLOT.get(g, ()):
                    ca = ga // JT
                    emit_av(ga, paccs[ca])
                    if DBG and ca == 0 and ga in (3, 7, 11):
                        pd = npool.tile([65, QC], F32, name="pd", tag="pd")
                        nc.vector.tensor_copy(pd[:], paccs[0][0][0:65, :])
                        nc.sync.dma_start(dbg_snap[ga // 4, :, :], pd[:])
                    if ga % JT == JT - 1:
                        emit_normalize(ca, paccs.pop(ca))

            # ---- tail: last chunk's output projection ----
            for tt in range(28, 32):
                emit_oproj(tt, use_act=True)
            if DBG:
                nc.sync.dma_start(dbg_q[:, :], qT[0][:])
                nc.sync.dma_start(dbg_k[:, :], kT[0][:])
                nc.sync.dma_start(
                    dbg_vb[:, :], v_b[0][:].bitcast(F32).rearrange("p a m -> p (a m)"))
    nc.finalize()
    return nc


@functools.cache
def _nc_cached():
    return build_nc()


def _prep_in_maps(inputs):
    np16 = np.float16

    def xbm(a):
        # [S,B,D] -> [D, B*S] batch-major tokens, fp16
        a = np.asarray(a, np.float32).transpose(2, 1, 0).reshape(D, T)
        return np.ascontiguousarray(a).astype(np16)

    xq_h = xbm(inputs["query"])
    xk_h = xbm(inputs["key"])
    xv_h = xbm(inputs["value"])
    Wq, Wk, Wv, Wo = (np.asarray(inputs[k], np.float32) for k in ("Wq", "Wk", "Wv", "Wo"))

    def wpack(w):
        # [D, FPC] (t p)-major -> [p=128, t*m] contiguous per partition
        return np.ascontiguousarray(
            w.reshape(DT, 128, FPC).transpose(1, 0, 2).reshape(128, DT * FPC)
        ).astype(np16)

    in_maps = []
    for c in range(NC):
        sl = slice(c * FPC, (c + 1) * FPC)
        in_maps.append({
            "xq": xq_h, "xk": xk_h, "xv": xv_h,
            "wq": wpack(Wq[sl, :].T),
            "wk": wpack(Wk[sl, :].T),
            "wv": wpack(Wv[sl, :].T),
            "wo": np.ascontiguousarray(Wo[:, sl].T).astype(np16),
        })
    return in_maps


def kernel(query, key, value, Wq, bq, Wk, bk, Wv, bv, Wo, bo):
    in_maps = _prep_in_maps({"query": query, "key": key, "value": value,
                             "Wq": Wq, "Wk": Wk, "Wv": Wv, "Wo": Wo})
    nc = _nc_cached()
    res = run_bass_kernel_spmd(nc, in_maps, core_ids=list(range(NC)))
    acc = np.zeros((T, D), np.float32)
    for r in res.results:
        acc += r["out"].astype(np.float32)
    acc += np.asarray(bo, np.float32)[None, :]
    # batch-major tokens back to [S, B, D]
    out = acc.reshape(B, S, D).transpose(1, 0, 2)
    out = np.ascontiguousarray(out, np.float32)
    for bias in (bq, bk, bv):
        assert float(np.abs(np.asarray(bias)).max()) == 0.0, "nonzero qkv bias unsupported"
    return out



# revision 14
# speedup vs baseline: 1.0867x; 1.0846x over previous
"""Multi-head attention (S=2048, B=2, D=1024, H=16) on 8 trn2 NeuronCores.

Sharding: 2 heads per core (head parallelism). Each core computes Q/K/V
projections for its 128 output features, attention for its 4 (batch,
head) pairs, and a partial output projection; the host sums the 8
partial outputs.

v4 restructure (from the 242us v3 trace): the TENSOR engine is the
bottleneck (~150us effective queue time), not the exp stream (~134us
busy), so the schedule keeps the tensor FIFO never-idle:
 - av matmuls and v_b/ej are bf16 (f32r moving operands streamed at
   half rate: 422ns -> 216ns per av matmul).
 - every 8-matmul projection group is split into 4+4 halves folded at
   adjacent slots so no fold bursts >1us delay the next scores pair
   (which gates exp via the 2-slot sj lead).
 - av(g) runs at slot g+1 (chunk 0: g+11, riding out the input DMA
   ramp) so an av never head-blocks the FIFO waiting on its exp.
 - consumption-ordered input DMA: k/q first-chunk halves, then k01..
   k03/v00..v03/q01 interleaved to match per-slot needs; weights ride
   the scalar HWDGE ring so the x stream owns the sync ring at t=0.
 - exp activation table preloaded at t~0 (dummy exp) to shave the
   2.7us ACT_TABLE_LOAD off the first real exp.
 - outputs: last two chunks on the (empty by then) sync HWDGE ring;
   earlier tiles trickle on the gpsimd SWDGE ring.
"""
import sys
sys.path.insert(0, '/opt/trn_rl_repo')
import functools
import os

import numpy as np

import concourse.bacc as bacc
import concourse.mybir as mybir
import concourse.tile as tile
from concourse.bass_utils import run_bass_kernel_spmd
from concourse.masks import make_identity

F32 = mybir.dt.float32
F32R = mybir.dt.float32r
F16 = mybir.dt.float16
BF16 = mybir.dt.bfloat16
AFT = mybir.ActivationFunctionType
MUL = mybir.AluOpType.mult

S, B, D, H = 2048, 2, 1024, 16
T = S * B               # 4096 tokens
DK = D // H             # 64
NC = 8                  # cores
FPC = D // NC           # 128 features per core (2 heads)
QC = 512                # q-chunk size
JT = S // 128           # 16 key tiles per batch
DT = D // 128           # 8 contraction tiles for projections
NCH = 8                 # chunks, b-outer: c -> b=c//4, qc=c%4
NSLOT = NCH * JT        # 128 global (chunk, key-tile) slots

EJ_DT = {"f32r": F32R, "bf16": BF16}[os.environ.get("EJ_DT", "bf16")]
OUT_DT = {"fp16": F16, "fp32": F32}[os.environ.get("OUT_DT", "fp16")]


def build_nc():
    nc = bacc.Bacc(None, target_bir_lowering=False)

    xq = nc.dram_tensor("xq", [D, T], F16, kind="ExternalInput")
    xk = nc.dram_tensor("xk", [D, T], F16, kind="ExternalInput")
    xv = nc.dram_tensor("xv", [D, T], F16, kind="ExternalInput")
    wq = nc.dram_tensor("wq", [128, DT * FPC], F16, kind="ExternalInput")
    wk = nc.dram_tensor("wk", [128, DT * FPC], F16, kind="ExternalInput")
    wv = nc.dram_tensor("wv", [128, DT * FPC], F16, kind="ExternalInput")
    wo = nc.dram_tensor("wo", [FPC, D], F16, kind="ExternalInput")
    out = nc.dram_tensor("out", [T, D], OUT_DT, kind="ExternalOutput")
    xsrc = {"q": xq, "k": xk, "v": xv}

    with tile.TileContext(nc) as tc:
        with (
            tc.tile_pool(name="wpool", bufs=1) as wpool,
            tc.tile_pool(name="proj", bufs=1) as projpool,
            tc.tile_pool(name="vtmp", bufs=2) as vtpool,
            tc.tile_pool(name="xdma", bufs=6) as xpool,
            tc.tile_pool(name="xh", bufs=4) as xhpool,
            tc.tile_pool(name="ej", bufs=26) as epool,
            tc.tile_pool(name="norm", bufs=2) as npool,
            tc.tile_pool(name="osb", bufs=3) as opool,
            tc.tile_pool(name="psS", bufs=2, space="PSUM") as psS,
            tc.tile_pool(name="psA", bufs=2, space="PSUM") as psA,
            tc.tile_pool(name="psM", bufs=2, space="PSUM") as psM,
        ):
            # ---- exp table preload: dummy exp at t~0 so the first real
            # exp doesn't pay the ~2.7us ACT_TABLE_LOAD.
            warm = npool.tile([1, 8], F32, name="warm", tag="warm")
            warm_o = npool.tile([1, 8], F32, name="warm_o", tag="warm_o")
            nc.vector.memset(warm[:], 0.0)
            nc.scalar.activation(warm_o[:], warm[:], AFT.Exp)

            # ---- weights / constants ----
            w_t = {}
            for name, wd in (("k", wk), ("q", wq), ("v", wv)):
                w_t[name] = wpool.tile([128, DT, FPC], F16, name=f"w_{name}")
                # pre-packed [128, DT*FPC] host-side: contiguous per
                # partition; scalar HWDGE ring so the x stream owns sync.
                nc.scalar.dma_start(w_t[name][:], wd.rearrange("p (t m) -> p t m", t=DT))
            ident = wpool.tile([128, 128], BF16, name="ident")
            make_identity(nc, ident[:])
            wo_t = wpool.tile([128, D], F16, name="wo_t")

            # ---- persistent activations ----
            kT = [projpool.tile([128, S], F16, name=f"kT{b}") for b in range(B)]
            qT = [projpool.tile([128, S], F16, name=f"qT{b}") for b in range(B)]
            v_b = [projpool.tile([128, JT, 130], BF16, name=f"v_b{b}") for b in range(B)]
            xT = projpool.tile([128, T], F16, name="xT")
            # ones columns of v_b (denominator trick) are static
            for b in range(B):
                for jt in range(JT):
                    nc.vector.memset(v_b[b][:, jt, 64:65], 1.0)
                    nc.vector.memset(v_b[b][:, jt, 129:130], 1.0)

            # ---- input DMA stream (emission order == transfer order) ----
            xt_tiles = {}
            xh_tiles = {}

            def emit_xdma(p, b, hf):
                t = xpool.tile([128, DT, 512], F16, name="xt", tag="xt")
                tok0 = b * S + hf * 512
                nc.sync.dma_start(
                    t[:],
                    xsrc[p].rearrange("(a p) m -> p a m", p=128)[:, :, tok0:tok0 + 512])
                xt_tiles[(p, b, hf)] = t

            def emit_xdma_half(p, half):
                t = xhpool.tile([128, DT, 256], F16, name="xth", tag="xth")
                tok0 = half * 256
                nc.sync.dma_start(
                    t[:],
                    xsrc[p].rearrange("(a p) m -> p a m", p=128)[:, :, tok0:tok0 + 256])
                xh_tiles[(p, half)] = t

            for p, half in (("k", 0), ("k", 1), ("q", 0), ("q", 1)):
                emit_xdma_half(p, half)
            dma_order = [
                ("k", 0, 1), ("v", 0, 0), ("k", 0, 2), ("v", 0, 1), ("k", 0, 3),
                ("v", 0, 2), ("q", 0, 1), ("v", 0, 3),
                "WO",
                ("q", 0, 2), ("k", 1, 0), ("k", 1, 1), ("k", 1, 2), ("k", 1, 3),
                ("q", 0, 3), ("v", 1, 0), ("v", 1, 1), ("v", 1, 2), ("v", 1, 3),
                ("q", 1, 0), ("q", 1, 1), ("q", 1, 2), ("q", 1, 3),
            ]
            for item in dma_order:
                if item == "WO":
                    nc.sync.dma_start(wo_t[:], wo[:, :])
                else:
                    emit_xdma(*item)

            # ---- building blocks ----
            def proj_half_tokens(p, half, dest):
                """full 8-matmul proj over a 256-token prologue half tile."""
                xt = xh_tiles.pop((p, half))
                ps = psM.tile([128, 256], F32, name="psph", tag="m")
                for dt in range(DT):
                    nc.tensor.matmul(ps[:], w_t[p][:, dt, :], xt[:, dt, :],
                                     start=(dt == 0), stop=(dt == DT - 1))
                nc.vector.tensor_copy(dest, ps[:])

            proj_ps = {}

            def proj_mm_half(p, b, hf, dest, part):
                """half (4 matmuls) of an 8-mm accumulation over 512 tokens."""
                key = (p, b, hf)
                if part == 0:
                    proj_ps[key] = psM.tile([128, 512], F32, name="psproj", tag="m")
                ps = proj_ps[key]
                xt = xt_tiles[key]
                for dt in range(4 * part, 4 * part + 4):
                    nc.tensor.matmul(ps[:], w_t[p][:, dt, :], xt[:, dt, :],
                                     start=(dt == 0), stop=(dt == DT - 1))
                if part == 1:
                    del xt_tiles[key], proj_ps[key]
                    nc.vector.tensor_copy(dest, ps[:])

            vstore = {}

            def v_proj_half(b, hf, part):
                if part == 0:
                    vstore[(b, hf)] = vtpool.tile([128, 512], BF16, name="vt", tag="vt")
                proj_mm_half("v", b, hf, vstore[(b, hf)][:], part)

            def v_tp(b, hf, jj):
                """transpose 2 of the 4 key-tiles of v half hf into v_b."""
                vt = vstore[(b, hf)]
                for u in range(2):
                    q = 2 * jj + u
                    jt = 4 * hf + q
                    tp = psM.tile([128, 128], BF16, name="tp", tag="m")
                    nc.tensor.transpose(
                        tp[:], vt[:, q * 128:(q + 1) * 128], ident[:])
                    nc.vector.tensor_copy(v_b[b][:, jt, 0:64], tp[:, 0:64])
                    nc.vector.tensor_copy(v_b[b][:, jt, 65:129], tp[:, 64:128])

            sj_t = {}
            ej_t = {}

            def emit_scores(g):
                c, j = divmod(g, JT)
                b, qc = divmod(c, 4)
                sj = psS.tile([128, 2, QC], F32, name="sj", tag="sj")
                for h in range(2):
                    nc.tensor.matmul(
                        sj[:, h, :], kT[b][h * 64:(h + 1) * 64, j * 128:(j + 1) * 128],
                        qT[b][h * 64:(h + 1) * 64, qc * QC:(qc + 1) * QC],
                        start=True, stop=True)
                sj_t[g] = sj

            def emit_exp(g):
                ej = epool.tile([128, 2, QC], EJ_DT, name="ej", tag="ej")
                nc.scalar.activation(ej[:], sj_t.pop(g)[:], AFT.Exp)
                ej_t[g] = ej

            def emit_av(g, pacc):
                c, j = divmod(g, JT)
                b = c // 4
                ej = ej_t.pop(g)
                for h in range(2):
                    nc.tensor.matmul(
                        pacc[h][0:65, :], v_b[b][:, j, h * 65:h * 65 + 65],
                        ej[:, h, :], start=(j == 0), stop=(j == JT - 1))

            def emit_normalize(c, pacc):
                for h in range(2):
                    dsb = npool.tile([1, QC], F32, name="dsb", tag="dsb")
                    nc.vector.tensor_copy(dsb[:], pacc[h][64:65, :])
                    rd = npool.tile([1, QC], F32, name="rd", tag="rd")
                    nc.vector.reciprocal_approx_fast(out=rd[:], in_=dsb[:])
                    bc = npool.tile([64, QC], F32, name="bc", tag="bc")
                    nc.gpsimd.partition_broadcast(bc[:], rd[:])
                    nc.vector.tensor_tensor(
                        out=xT[h * 64:(h + 1) * 64, c * QC:(c + 1) * QC],
                        in0=pacc[h][0:64, :], in1=bc[:], op=MUL)

            def emit_oproj(tt, use_act=False):
                osb = opool.tile([128, D], OUT_DT, name="osb", tag="osb")
                for ec in range(2):
                    po = psM.tile([128, 512], F32, name="po", tag="m")
                    nc.tensor.matmul(po[:], xT[:, tt * 128:(tt + 1) * 128],
                                     wo_t[:, ec * 512:(ec + 1) * 512],
                                     start=True, stop=True)
                    if use_act and ec == 0:
                        nc.scalar.copy(osb[:, 0:512], po[:])
                    else:
                        nc.vector.tensor_copy(osb[:, ec * 512:(ec + 1) * 512], po[:])
                # last two chunks' outputs ride the (empty by then) sync
                # HWDGE ring at full rate; earlier tiles trickle on SWDGE.
                eng = nc.sync if tt >= 24 else nc.gpsimd
                eng.dma_start(out[tt * 128:(tt + 1) * 128, :], osb[:])

            # ---- fold schedule (each item <=~1us of PE) ----
            def k_half(hf, part):
                return lambda: proj_mm_half(
                    "k", 0, hf, kT[0][:, hf * 512:(hf + 1) * 512], part)

            def k1_half(hf, part):
                return lambda: proj_mm_half(
                    "k", 1, hf, kT[1][:, hf * 512:(hf + 1) * 512], part)

            def q_half(c, part):
                b, qc = divmod(c, 4)
                return lambda: proj_mm_half(
                    "q", b, qc, qT[b][:, qc * 512:(qc + 1) * 512], part)

            def vp_half(b, hf, part):
                return lambda: v_proj_half(b, hf, part)

            def vt_item(b, hf, jj):
                return lambda: v_tp(b, hf, jj)

            def op_item(tt):
                return lambda: emit_oproj(tt)

            FOLD = {}

            def put(g, th):
                FOLD.setdefault(g, []).append(th)

            def put_pair(g0, mk, *args):
                put(g0, mk(*args, 0))
                put(g0 + 1, mk(*args, 1))

            # b0 projections + v prep, arrival-matched to the DMA stream
            put_pair(0, k_half, 1)
            put_pair(2, vp_half, 0, 0)
            put_pair(4, k_half, 2)
            put(6, vt_item(0, 0, 0)); put(6, vt_item(0, 0, 1))
            put_pair(7, vp_half, 0, 1)
            put_pair(8, k_half, 3)
            put(11, vt_item(0, 1, 0)); put(11, vt_item(0, 1, 1))
            put_pair(12, q_half, 1)
            put_pair(14, vp_half, 0, 2)
            put(16, vt_item(0, 2, 0)); put(16, vt_item(0, 2, 1))
            put_pair(17, vp_half, 0, 3)
            put(19, vt_item(0, 3, 0)); put(19, vt_item(0, 3, 1))
            put_pair(26, q_half, 2)
            # b1 prep under b0's chunks 2-3
            put_pair(36, k1_half, 0)
            put_pair(38, k1_half, 1)
            put_pair(40, k1_half, 2)
            put_pair(42, k1_half, 3)
            put_pair(44, q_half, 3)
            put_pair(46, vp_half, 1, 0)
            put(48, vt_item(1, 0, 0)); put(48, vt_item(1, 0, 1))
            put_pair(49, vp_half, 1, 1)
            put(51, vt_item(1, 1, 0)); put(51, vt_item(1, 1, 1))
            put_pair(52, vp_half, 1, 2)
            put(54, vt_item(1, 2, 0)); put(54, vt_item(1, 2, 1))
            put_pair(55, vp_half, 1, 3)
            put(57, vt_item(1, 3, 0)); put(57, vt_item(1, 3, 1))
            put_pair(58, q_half, 4)
            put_pair(74, q_half, 5)
            put_pair(90, q_half, 6)
            put_pair(106, q_half, 7)
            # output projection tiles, spread into fold-free slots
            op_slots = {0: (29, 31, 33, 35), 1: (67, 69, 71, 73),
                        2: (66, 68, 70, 72), 3: (77, 79, 81, 83),
                        4: (93, 95, 97, 99), 5: (109, 111, 113, 115),
                        6: (117, 119, 121, 123)}
            for c, slots in op_slots.items():
                for i, g in enumerate(slots):
                    put(g, op_item(4 * c + i))

            # per-chunk av lag: chunk 0 rides out the input DMA ramp
            AV_LAG = {c: (11 if c == 0 else 1) for c in range(NCH)}
            AV_AT = {}
            for g in range(NSLOT):
                AV_AT.setdefault(g + AV_LAG[g // JT], []).append(g)

            # ---- prologue: first k/q chunk from 256-token halves ----
            proj_half_tokens("k", 0, kT[0][:, 0:256])
            proj_half_tokens("k", 1, kT[0][:, 256:512])
            proj_half_tokens("q", 0, qT[0][:, 0:256])
            proj_half_tokens("q", 1, qT[0][:, 256:512])
            emit_scores(0)
            emit_scores(1)

            # ---- the stream ----
            paccs = {}

            def run_avs(slot):
                for ga in AV_AT.get(slot, ()):
                    ca = ga // JT
                    if ga % JT == 0:
                        paccs[ca] = [psA.tile([128, QC], F32, name=f"pacc{h}",
                                              tag="pacc") for h in range(2)]
                    emit_av(ga, paccs[ca])
                    if ga % JT == JT - 1:
                        emit_normalize(ca, paccs.pop(ca))

            for g in range(NSLOT):
                emit_exp(g)
                if g + 2 < NSLOT:
                    emit_scores(g + 2)
                for th in FOLD.get(g, ()):
                    th()
                run_avs(g)

            # ---- tail: trailing avs, then last chunk's output projection ----
            for slot in range(NSLOT, NSLOT + max(AV_LAG.values()) + 1):
                run_avs(slot)
            for tt in range(28, 32):
                emit_oproj(tt, use_act=True)
    nc.finalize()
    return nc


@functools.cache
def _nc_cached():
    return build_nc()


def _prep_in_maps(inputs):
    np16 = np.float16

    def xbm(a):
        # [S,B,D] -> [D, B*S] batch-major tokens, fp16
        a = np.asarray(a, np.float32).transpose(2, 1, 0).reshape(D, T)
        return np.ascontiguousarray(a).astype(np16)

    xq_h = xbm(inputs["query"])
    xk_h = xbm(inputs["key"])
    xv_h = xbm(inputs["value"])
    Wq, Wk, Wv, Wo = (np.asarray(inputs[k], np.float32) for k in ("Wq", "Wk", "Wv", "Wo"))

    def wpack(w):
        # [D, FPC] (t p)-major -> [p=128, t*m] contiguous per partition
        return np.ascontiguousarray(
            w.reshape(DT, 128, FPC).transpose(1, 0, 2).reshape(128, DT * FPC)
        ).astype(np16)

    in_maps = []
    for c in range(NC):
        sl = slice(c * FPC, (c + 1) * FPC)
        in_maps.append({
            "xq": xq_h, "xk": xk_h, "xv": xv_h,
            "wq": wpack(Wq[sl, :].T),
            "wk": wpack(Wk[sl, :].T),
            "wv": wpack(Wv[sl, :].T),
            "wo": np.ascontiguousarray(Wo[:, sl].T).astype(np16),
        })
    return in_maps


def kernel(query, key, value, Wq, bq, Wk, bk, Wv, bv, Wo, bo):
    in_maps = _prep_in_maps({"query": query, "key": key, "value": value,
                             "Wq": Wq, "Wk": Wk, "Wv": Wv, "Wo": Wo})
    nc = _nc_cached()
    res = run_bass_kernel_spmd(nc, in_maps, core_ids=list(range(NC)))
    acc = np.zeros((T, D), np.float32)
    for r in res.results:
        acc += r["out"].astype(np.float32)
    acc += np.asarray(bo, np.float32)[None, :]
    # batch-major tokens back to [S, B, D]
    out = acc.reshape(B, S, D).transpose(1, 0, 2)
    out = np.ascontiguousarray(out, np.float32)
    for bias in (bq, bk, bv):
        assert float(np.abs(np.asarray(bias)).max()) == 0.0, "nonzero qkv bias unsupported"
    return out


# revision 19
# speedup vs baseline: 1.1036x; 1.0156x over previous
"""Multi-head attention (S=2048, B=2, D=1024, H=16) on 8 trn2 NeuronCores.

Sharding: 2 heads per core (head parallelism). Each core computes Q/K/V
projections for its 128 output features, attention for its 4 (batch,
head) pairs, and a partial output projection; the host sums the 8
partial outputs.

v4 restructure (from the 242us v3 trace): the TENSOR engine is the
bottleneck (~150us effective queue time), not the exp stream (~134us
busy), so the schedule keeps the tensor FIFO never-idle:
 - av matmuls and v_b/ej are bf16 (f32r moving operands streamed at
   half rate: 422ns -> 216ns per av matmul).
 - every 8-matmul projection group is split into 4+4 halves folded at
   adjacent slots so no fold bursts >1us delay the next scores pair
   (which gates exp via the 2-slot sj lead).
 - av(g) runs at slot g+1 (chunk 0: g+11, riding out the input DMA
   ramp) so an av never head-blocks the FIFO waiting on its exp.
 - consumption-ordered input DMA: k/q first-chunk halves, then k01..
   k03/v00..v03/q01 interleaved to match per-slot needs; weights ride
   the scalar HWDGE ring so the x stream owns the sync ring at t=0.
 - exp activation table preloaded at t~0 (dummy exp) to shave the
   2.7us ACT_TABLE_LOAD off the first real exp.
 - outputs: last two chunks on the (empty by then) sync HWDGE ring;
   earlier tiles trickle on the gpsimd SWDGE ring.
"""
import sys
sys.path.insert(0, '/opt/trn_rl_repo')
import functools
import os

import numpy as np

import concourse.bacc as bacc
import concourse.mybir as mybir
import concourse.tile as tile
from concourse.bass_utils import run_bass_kernel_spmd
from concourse.masks import make_identity

F32 = mybir.dt.float32
F32R = mybir.dt.float32r
F16 = mybir.dt.float16
BF16 = mybir.dt.bfloat16
AFT = mybir.ActivationFunctionType
MUL = mybir.AluOpType.mult

S, B, D, H = 2048, 2, 1024, 16
T = S * B               # 4096 tokens
DK = D // H             # 64
NC = 8                  # cores
FPC = D // NC           # 128 features per core (2 heads)
QC = 512                # q-chunk size
JT = S // 128           # 16 key tiles per batch
DT = D // 128           # 8 contraction tiles for projections
NCH = 8                 # chunks, b-outer: c -> b=c//4, qc=c%4
NSLOT = NCH * JT        # 128 global (chunk, key-tile) slots

EJ_DT = {"f32r": F32R, "bf16": BF16}[os.environ.get("EJ_DT", "bf16")]
OUT_DT = {"fp16": F16, "fp32": F32}[os.environ.get("OUT_DT", "fp16")]


def build_nc():
    nc = bacc.Bacc(None, target_bir_lowering=False)

    # x pre-arranged host-side as [128, (b hf) a m]: every 512-token tile is
    # contiguous per partition -> 128 DMA descriptors per tile, not 1024.
    XW = B * 4 * DT * 512
    xq = nc.dram_tensor("xq", [128, XW], F16, kind="ExternalInput")
    xk = nc.dram_tensor("xk", [128, XW], F16, kind="ExternalInput")
    xv = nc.dram_tensor("xv", [128, XW], F16, kind="ExternalInput")
    wq = nc.dram_tensor("wq", [128, DT * FPC], F16, kind="ExternalInput")
    wk = nc.dram_tensor("wk", [128, DT * FPC], F16, kind="ExternalInput")
    wv = nc.dram_tensor("wv", [128, DT * FPC], F16, kind="ExternalInput")
    wo = nc.dram_tensor("wo", [FPC, D], F16, kind="ExternalInput")
    out = nc.dram_tensor("out", [T, D], OUT_DT, kind="ExternalOutput")
    xsrc = {"q": xq, "k": xk, "v": xv}

    with tile.TileContext(nc) as tc:
        with (
            tc.tile_pool(name="wpool", bufs=1) as wpool,
            tc.tile_pool(name="proj", bufs=1) as projpool,
            tc.tile_pool(name="vtmp", bufs=2) as vtpool,
            tc.tile_pool(name="xdma", bufs=6) as xpool,
            tc.tile_pool(name="xh", bufs=4) as xhpool,
            tc.tile_pool(name="ej", bufs=26) as epool,
            tc.tile_pool(name="norm", bufs=2) as npool,
            tc.tile_pool(name="osb", bufs=3) as opool,
            tc.tile_pool(name="psS", bufs=2, space="PSUM") as psS,
            tc.tile_pool(name="psA", bufs=2, space="PSUM") as psA,
            tc.tile_pool(name="psM", bufs=2, space="PSUM") as psM,
        ):
            # ---- exp table preload: dummy exp at t~0 so the first real
            # exp doesn't pay the ~2.7us ACT_TABLE_LOAD.
            warm = npool.tile([1, 8], F32, name="warm", tag="warm")
            warm_o = npool.tile([1, 8], F32, name="warm_o", tag="warm_o")
            nc.vector.memset(warm[:], 0.0)
            nc.scalar.activation(warm_o[:], warm[:], AFT.Exp)

            # ---- weights / constants ----
            w_t = {}
            for name, wd in (("k", wk), ("q", wq), ("v", wv)):
                w_t[name] = wpool.tile([128, DT, FPC], F16, name=f"w_{name}")
                # pre-packed [128, DT*FPC] host-side: contiguous per
                # partition; scalar HWDGE ring so the x stream owns sync.
                nc.scalar.dma_start(w_t[name][:], wd.rearrange("p (t m) -> p t m", t=DT))
            ident = wpool.tile([128, 128], BF16, name="ident")
            make_identity(nc, ident[:])
            wo_t = wpool.tile([128, D], F16, name="wo_t")

            # ---- persistent activations ----
            kT = [projpool.tile([128, S], F16, name=f"kT{b}") for b in range(B)]
            qT = [projpool.tile([128, S], F16, name=f"qT{b}") for b in range(B)]
            v_b = [projpool.tile([128, JT, 130], BF16, name=f"v_b{b}") for b in range(B)]
            xT = projpool.tile([128, T], F16, name="xT")
            # ones columns of v_b (denominator trick) are static
            for b in range(B):
                for jt in range(JT):
                    nc.vector.memset(v_b[b][:, jt, 64:65], 1.0)
                    nc.vector.memset(v_b[b][:, jt, 129:130], 1.0)

            # ---- input DMA stream (emission order == transfer order) ----
            xt_tiles = {}
            xh_tiles = {}

            def emit_xdma(p, b, hf):
                t = xpool.tile([128, DT, 512], F16, name="xt", tag="xt")
                idx = b * 4 + hf
                nc.sync.dma_start(
                    t[:],
                    xsrc[p][:, idx * DT * 512:(idx + 1) * DT * 512]
                    .rearrange("p (a m) -> p a m", a=DT))
                xt_tiles[(p, b, hf)] = t

            def emit_xdma_half(p, half):
                t = xhpool.tile([128, DT, 256], F16, name="xth", tag="xth")
                nc.sync.dma_start(
                    t[:],
                    xsrc[p][:, 0:DT * 512]
                    .rearrange("p (a m) -> p a m", a=DT)[:, :, half * 256:(half + 1) * 256])
                xh_tiles[(p, half)] = t

            for p, half in (("k", 0), ("k", 1), ("q", 0), ("q", 1)):
                emit_xdma_half(p, half)
            dma_order = [
                ("k", 0, 1), ("v", 0, 0), ("k", 0, 2), ("v", 0, 1), ("k", 0, 3),
                ("v", 0, 2), ("q", 0, 1), ("v", 0, 3),
                "WO",
                ("q", 0, 2), ("k", 1, 0), ("k", 1, 1), ("k", 1, 2), ("k", 1, 3),
                ("q", 0, 3), ("v", 1, 0), ("v", 1, 1), ("v", 1, 2), ("v", 1, 3),
                ("q", 1, 0), ("q", 1, 1), ("q", 1, 2), ("q", 1, 3),
            ]
            for item in dma_order:
                if item == "WO":
                    nc.sync.dma_start(wo_t[:], wo[:, :])
                else:
                    emit_xdma(*item)

            # ---- building blocks ----
            def proj_half_tokens(p, half, dest):
                """full 8-matmul proj over a 256-token prologue half tile."""
                xt = xh_tiles.pop((p, half))
                ps = psM.tile([128, 256], F32, name="psph", tag="m")
                for dt in range(DT):
                    nc.tensor.matmul(ps[:], w_t[p][:, dt, :], xt[:, dt, :],
                                     start=(dt == 0), stop=(dt == DT - 1))
                nc.vector.tensor_copy(dest, ps[:])

            proj_ps = {}

            def proj_mm_half(p, b, hf, dest, part):
                """half (4 matmuls) of an 8-mm accumulation over 512 tokens."""
                key = (p, b, hf)
                if part == 0:
                    proj_ps[key] = psM.tile([128, 512], F32, name="psproj", tag="m")
                ps = proj_ps[key]
                xt = xt_tiles[key]
                for dt in range(4 * part, 4 * part + 4):
                    nc.tensor.matmul(ps[:], w_t[p][:, dt, :], xt[:, dt, :],
                                     start=(dt == 0), stop=(dt == DT - 1))
                if part == 1:
                    del xt_tiles[key], proj_ps[key]
                    nc.vector.tensor_copy(dest, ps[:])

            vstore = {}

            def v_proj_half(b, hf, part):
                if part == 0:
                    vstore[(b, hf)] = vtpool.tile([128, 512], BF16, name="vt", tag="vt")
                proj_mm_half("v", b, hf, vstore[(b, hf)][:], part)

            def v_tp(b, hf, jj):
                """transpose 2 of the 4 key-tiles of v half hf into v_b."""
                vt = vstore[(b, hf)]
                for u in range(2):
                    q = 2 * jj + u
                    jt = 4 * hf + q
                    tp = psM.tile([128, 128], BF16, name="tp", tag="m")
                    nc.tensor.transpose(
                        tp[:], vt[:, q * 128:(q + 1) * 128], ident[:])
                    nc.vector.tensor_copy(v_b[b][:, jt, 0:64], tp[:, 0:64])
                    nc.vector.tensor_copy(v_b[b][:, jt, 65:129], tp[:, 64:128])

            sj_t = {}
            ej_t = {}

            def emit_scores(g):
                c, j = divmod(g, JT)
                b, qc = divmod(c, 4)
                sj = psS.tile([128, 2, QC], F32, name="sj", tag="sj")
                for h in range(2):
                    nc.tensor.matmul(
                        sj[:, h, :], kT[b][h * 64:(h + 1) * 64, j * 128:(j + 1) * 128],
                        qT[b][h * 64:(h + 1) * 64, qc * QC:(qc + 1) * QC],
                        start=True, stop=True)
                sj_t[g] = sj

            def emit_exp(g):
                ej = epool.tile([128, 2, QC], EJ_DT, name="ej", tag="ej")
                nc.scalar.activation(ej[:], sj_t.pop(g)[:], AFT.Exp)
                ej_t[g] = ej

            def emit_av(g, pacc):
                c, j = divmod(g, JT)
                b = c // 4
                ej = ej_t.pop(g)
                for h in range(2):
                    nc.tensor.matmul(
                        pacc[h][0:65, :], v_b[b][:, j, h * 65:h * 65 + 65],
                        ej[:, h, :], start=(j == 0), stop=(j == JT - 1))

            def emit_normalize_h(c, pacc, h):
                dsb = npool.tile([1, QC], F32, name="dsb", tag="dsb")
                nc.vector.tensor_copy(dsb[:], pacc[h][64:65, :])
                rd = npool.tile([1, QC], F32, name="rd", tag="rd")
                nc.vector.reciprocal_approx_fast(out=rd[:], in_=dsb[:])
                bc = npool.tile([64, QC], F32, name="bc", tag="bc")
                nc.gpsimd.partition_broadcast(bc[:], rd[:])
                nc.vector.tensor_tensor(
                    out=xT[h * 64:(h + 1) * 64, c * QC:(c + 1) * QC],
                    in0=pacc[h][0:64, :], in1=bc[:], op=MUL)

            def emit_oproj(tt, use_act=False):
                osb = opool.tile([128, D], OUT_DT, name="osb", tag="osb")
                for ec in range(2):
                    po = psM.tile([128, 512], F32, name="po", tag="m")
                    nc.tensor.matmul(po[:], xT[:, tt * 128:(tt + 1) * 128],
                                     wo_t[:, ec * 512:(ec + 1) * 512],
                                     start=True, stop=True)
                    if use_act and ec == 0:
                        nc.scalar.copy(osb[:, 0:512], po[:])
                    else:
                        nc.vector.tensor_copy(osb[:, ec * 512:(ec + 1) * 512], po[:])
                # last two chunks' outputs ride the (empty by then) sync
                # HWDGE ring at full rate; earlier tiles trickle on SWDGE.
                eng = nc.sync if tt >= 24 else nc.gpsimd
                eng.dma_start(out[tt * 128:(tt + 1) * 128, :], osb[:])

            # ---- fold schedule (each item <=~1us of PE) ----
            def k_half(hf, part):
                return lambda: proj_mm_half(
                    "k", 0, hf, kT[0][:, hf * 512:(hf + 1) * 512], part)

            def k1_half(hf, part):
                return lambda: proj_mm_half(
                    "k", 1, hf, kT[1][:, hf * 512:(hf + 1) * 512], part)

            def q_half(c, part):
                b, qc = divmod(c, 4)
                return lambda: proj_mm_half(
                    "q", b, qc, qT[b][:, qc * 512:(qc + 1) * 512], part)

            def vp_half(b, hf, part):
                return lambda: v_proj_half(b, hf, part)

            def vt_item(b, hf, jj):
                return lambda: v_tp(b, hf, jj)

            def op_item(tt):
                return lambda: emit_oproj(tt)

            FOLD = {}

            def put(g, th):
                FOLD.setdefault(g, []).append(th)

            def put_pair(g0, mk, *args):
                put(g0, mk(*args, 0))
                put(g0 + 1, mk(*args, 1))

            # b0 projections + v prep, arrival-matched to the DMA stream
            put_pair(0, k_half, 1)
            put_pair(2, vp_half, 0, 0)
            put_pair(4, k_half, 2)
            put(6, vt_item(0, 0, 0)); put(6, vt_item(0, 0, 1))
            put_pair(7, vp_half, 0, 1)
            put_pair(8, k_half, 3)
            put(11, vt_item(0, 1, 0)); put(11, vt_item(0, 1, 1))
            put_pair(12, q_half, 1)
            put_pair(14, vp_half, 0, 2)
            put(16, vt_item(0, 2, 0)); put(16, vt_item(0, 2, 1))
            put_pair(17, vp_half, 0, 3)
            put(19, vt_item(0, 3, 0)); put(19, vt_item(0, 3, 1))
            put_pair(26, q_half, 2)
            # b1 prep under b0's chunks 2-3
            put_pair(36, k1_half, 0)
            put_pair(38, k1_half, 1)
            put_pair(40, k1_half, 2)
            put_pair(42, k1_half, 3)
            put_pair(44, q_half, 3)
            put_pair(46, vp_half, 1, 0)
            put(48, vt_item(1, 0, 0)); put(48, vt_item(1, 0, 1))
            put_pair(49, vp_half, 1, 1)
            put(51, vt_item(1, 1, 0)); put(51, vt_item(1, 1, 1))
            put_pair(52, vp_half, 1, 2)
            put(54, vt_item(1, 2, 0)); put(54, vt_item(1, 2, 1))
            put_pair(55, vp_half, 1, 3)
            put(57, vt_item(1, 3, 0)); put(57, vt_item(1, 3, 1))
            put_pair(58, q_half, 4)
            put_pair(74, q_half, 5)
            put_pair(90, q_half, 6)
            put_pair(106, q_half, 7)
            # output projection tiles, spread into fold-free slots
            op_slots = {0: (29, 31, 33, 35), 1: (67, 69, 71, 73),
                        2: (66, 68, 70, 72), 3: (77, 79, 81, 83),
                        4: (93, 95, 97, 99), 5: (109, 111, 113, 115),
                        6: (117, 119, 121, 123)}
            for c, slots in op_slots.items():
                for i, g in enumerate(slots):
                    put(g, op_item(4 * c + i))

            # per-chunk av lag: chunk 0 rides out the input DMA ramp
            AV_LAG = {c: (11 if c == 0 else 1) for c in range(NCH)}
            AV_AT = {}
            for g in range(NSLOT):
                AV_AT.setdefault(g + AV_LAG[g // JT], []).append(g)

            # ---- prologue: first k/q chunk from 256-token halves ----
            proj_half_tokens("k", 0, kT[0][:, 0:256])
            proj_half_tokens("k", 1, kT[0][:, 256:512])
            proj_half_tokens("q", 0, qT[0][:, 0:256])
            proj_half_tokens("q", 1, qT[0][:, 256:512])
            emit_scores(0)
            emit_scores(1)

            # ---- the stream ----
            paccs = {}

            norm_pend = {}

            def run_avs(slot):
                # second normalize half from the previous chunk, one slot
                # later, so the vector burst at a chunk boundary is halved
                for ca, pacc in list(norm_pend.items()):
                    emit_normalize_h(ca, pacc, 1)
                    del norm_pend[ca]
                for ga in AV_AT.get(slot, ()):
                    ca = ga // JT
                    if ga % JT == 0:
                        paccs[ca] = [psA.tile([128, QC], F32, name=f"pacc{h}",
                                              tag="pacc") for h in range(2)]
                    emit_av(ga, paccs[ca])
                    if ga % JT == JT - 1:
                        pacc = paccs.pop(ca)
                        emit_normalize_h(ca, pacc, 0)
                        norm_pend[ca] = pacc

            for g in range(NSLOT):
                emit_exp(g)
                if g + 2 < NSLOT:
                    emit_scores(g + 2)
                for th in FOLD.get(g, ()):
                    th()
                run_avs(g)

            # ---- tail: trailing avs, then last chunk's output projection ----
            for slot in range(NSLOT, NSLOT + max(AV_LAG.values()) + 1):
                run_avs(slot)
            for tt in range(28, 32):
                emit_oproj(tt, use_act=True)
    nc.finalize()
    return nc


@functools.cache
def _nc_cached():
    return build_nc()


def _prep_in_maps(inputs):
    np16 = np.float16

    def xbm(a):
        # [S,B,D] -> [128, (b hf) a m]: tile-contiguous per partition
        xd = np.asarray(a, np.float32).transpose(2, 1, 0)        # [D, B, S]
        xd = xd.reshape(DT, 128, B, 4, 512).transpose(1, 2, 3, 0, 4)
        return np.ascontiguousarray(xd.reshape(128, B * 4 * DT * 512)).astype(np16)

    xq_h = xbm(inputs["query"])
    xk_h = xbm(inputs["key"])
    xv_h = xbm(inputs["value"])
    Wq, Wk, Wv, Wo = (np.asarray(inputs[k], np.float32) for k in ("Wq", "Wk", "Wv", "Wo"))

    def wpack(w):
        # [D, FPC] (t p)-major -> [p=128, t*m] contiguous per partition
        return np.ascontiguousarray(
            w.reshape(DT, 128, FPC).transpose(1, 0, 2).reshape(128, DT * FPC)
        ).astype(np16)

    in_maps = []
    for c in range(NC):
        sl = slice(c * FPC, (c + 1) * FPC)
        in_maps.append({
            "xq": xq_h, "xk": xk_h, "xv": xv_h,
            "wq": wpack(Wq[sl, :].T),
            "wk": wpack(Wk[sl, :].T),
            "wv": wpack(Wv[sl, :].T),
            "wo": np.ascontiguousarray(Wo[:, sl].T).astype(np16),
        })
    return in_maps


def kernel(query, key, value, Wq, bq, Wk, bk, Wv, bv, Wo, bo):
    in_maps = _prep_in_maps({"query": query, "key": key, "value": value,
                             "Wq": Wq, "Wk": Wk, "Wv": Wv, "Wo": Wo})
    nc = _nc_cached()
    res = run_bass_kernel_spmd(nc, in_maps, core_ids=list(range(NC)))
    acc = np.zeros((T, D), np.float32)
    for r in res.results:
        acc += r["out"].astype(np.float32)
    acc += np.asarray(bo, np.float32)[None, :]
    # batch-major tokens back to [S, B, D]
    out = acc.reshape(B, S, D).transpose(1, 0, 2)
    out = np.ascontiguousarray(out, np.float32)
    for bias in (bq, bk, bv):
        assert float(np.abs(np.asarray(bias)).max()) == 0.0, "nonzero qkv bias unsupported"
    return out


# revision 26
# speedup vs baseline: 1.1084x; 1.0043x over previous
"""Multi-head attention (S=2048, B=2, D=1024, H=16) on 8 trn2 NeuronCores.

Sharding: 2 heads per core (head parallelism). Each core computes Q/K/V
projections for its 128 output features, attention for its 4 (batch,
head) pairs, and a partial output projection; the host sums the 8
partial outputs.

v4 restructure (from the 242us v3 trace): the TENSOR engine is the
bottleneck (~150us effective queue time), not the exp stream (~134us
busy), so the schedule keeps the tensor FIFO never-idle:
 - av matmuls and v_b/ej are bf16 (f32r moving operands streamed at
   half rate: 422ns -> 216ns per av matmul).
 - every 8-matmul projection group is split into 4+4 halves folded at
   adjacent slots so no fold bursts >1us delay the next scores pair
   (which gates exp via the 2-slot sj lead).
 - av(g) runs at slot g+1 (chunk 0: g+11, riding out the input DMA
   ramp) so an av never head-blocks the FIFO waiting on its exp.
 - consumption-ordered input DMA: k/q first-chunk halves, then k01..
   k03/v00..v03/q01 interleaved to match per-slot needs; weights ride
   the scalar HWDGE ring so the x stream owns the sync ring at t=0.
 - exp activation table preloaded at t~0 (dummy exp) to shave the
   2.7us ACT_TABLE_LOAD off the first real exp.
 - outputs: last two chunks on the (empty by then) sync HWDGE ring;
   earlier tiles trickle on the gpsimd SWDGE ring.
"""
import sys
sys.path.insert(0, '/opt/trn_rl_repo')
import functools
import os

import numpy as np

import concourse.bacc as bacc
import concourse.mybir as mybir
import concourse.tile as tile
from concourse.bass_utils import run_bass_kernel_spmd
from concourse.masks import make_identity

F32 = mybir.dt.float32
F32R = mybir.dt.float32r
F16 = mybir.dt.float16
BF16 = mybir.dt.bfloat16
AFT = mybir.ActivationFunctionType
MUL = mybir.AluOpType.mult

S, B, D, H = 2048, 2, 1024, 16
T = S * B               # 4096 tokens
DK = D // H             # 64
NC = 8                  # cores
FPC = D // NC           # 128 features per core (2 heads)
QC = 512                # q-chunk size
JT = S // 128           # 16 key tiles per batch
DT = D // 128           # 8 contraction tiles for projections
NCH = 8                 # chunks, b-outer: c -> b=c//4, qc=c%4
NSLOT = NCH * JT        # 128 global (chunk, key-tile) slots

EJ_DT = {"f32r": F32R, "bf16": BF16}[os.environ.get("EJ_DT", "bf16")]
OUT_DT = {"fp16": F16, "fp32": F32}[os.environ.get("OUT_DT", "fp16")]


def build_nc():
    nc = bacc.Bacc(None, target_bir_lowering=False)

    # x pre-arranged host-side as [128, (b hf) a m]: every 512-token tile is
    # contiguous per partition -> 128 DMA descriptors per tile, not 1024.
    XW = B * 4 * DT * 512
    xq = nc.dram_tensor("xq", [128, XW], F16, kind="ExternalInput")
    xk = nc.dram_tensor("xk", [128, XW], F16, kind="ExternalInput")
    xv = nc.dram_tensor("xv", [128, XW], F16, kind="ExternalInput")
    wq = nc.dram_tensor("wq", [128, DT * FPC], F16, kind="ExternalInput")
    wk = nc.dram_tensor("wk", [128, DT * FPC], F16, kind="ExternalInput")
    wv = nc.dram_tensor("wv", [128, DT * FPC], F16, kind="ExternalInput")
    wo = nc.dram_tensor("wo", [FPC, D], F16, kind="ExternalInput")
    out = nc.dram_tensor("out", [T, D], OUT_DT, kind="ExternalOutput")
    xsrc = {"q": xq, "k": xk, "v": xv}

    with tile.TileContext(nc) as tc:
        with (
            tc.tile_pool(name="wpool", bufs=1) as wpool,
            tc.tile_pool(name="proj", bufs=1) as projpool,
            tc.tile_pool(name="vtmp", bufs=2) as vtpool,
            tc.tile_pool(name="xdma", bufs=8) as xpool,
            tc.tile_pool(name="ej", bufs=26) as epool,
            tc.tile_pool(name="norm", bufs=2) as npool,
            tc.tile_pool(name="osb", bufs=3) as opool,
            tc.tile_pool(name="psS", bufs=2, space="PSUM") as psS,
            tc.tile_pool(name="psA", bufs=2, space="PSUM") as psA,
            tc.tile_pool(name="psM", bufs=2, space="PSUM") as psM,
        ):
            # ---- exp table preload: dummy exp at t~0 so the first real
            # exp doesn't pay the ~2.7us ACT_TABLE_LOAD.
            warm = npool.tile([1, 8], F32, name="warm", tag="warm")
            warm_o = npool.tile([1, 8], F32, name="warm_o", tag="warm_o")
            nc.vector.memset(warm[:], 0.0)
            nc.scalar.activation(warm_o[:], warm[:], AFT.Exp)

            # ---- weights / constants ----
            w_t = {}
            for name, wd in (("k", wk), ("q", wq), ("v", wv)):
                w_t[name] = wpool.tile([128, DT, FPC], F16, name=f"w_{name}")
                # pre-packed [128, DT*FPC] host-side: contiguous per
                # partition; scalar HWDGE ring so the x stream owns sync.
                nc.scalar.dma_start(w_t[name][:], wd.rearrange("p (t m) -> p t m", t=DT))
            ident = wpool.tile([128, 128], BF16, name="ident")
            make_identity(nc, ident[:])
            wo_t = wpool.tile([128, D], F16, name="wo_t")

            # ---- persistent activations ----
            kT = [projpool.tile([128, S], F16, name=f"kT{b}") for b in range(B)]
            qT = [projpool.tile([128, S], F16, name=f"qT{b}") for b in range(B)]
            v_b = [projpool.tile([128, JT, 130], BF16, name=f"v_b{b}") for b in range(B)]
            xT = projpool.tile([128, T], F16, name="xT")
            # ones columns of v_b (denominator trick) are static
            for b in range(B):
                for jt in range(JT):
                    nc.vector.memset(v_b[b][:, jt, 64:65], 1.0)
                    nc.vector.memset(v_b[b][:, jt, 129:130], 1.0)

            # ---- input DMA stream (emission order == transfer order) ----
            xt_tiles = {}
            xh_tiles = {}

            def emit_xdma(p, b, hf):
                t = xpool.tile([128, DT, 512], F16, name="xt", tag="xt")
                idx = b * 4 + hf
                nc.sync.dma_start(
                    t[:],
                    xsrc[p][:, idx * DT * 512:(idx + 1) * DT * 512]
                    .rearrange("p (a m) -> p a m", a=DT))
                xt_tiles[(p, b, hf)] = t

            emit_xdma("k", 0, 0)
            emit_xdma("q", 0, 0)
            dma_order = [
                ("k", 0, 1), ("v", 0, 0), ("k", 0, 2), ("v", 0, 1), ("k", 0, 3),
                ("v", 0, 2), ("q", 0, 1), ("v", 0, 3),
                "WO",
                ("q", 0, 2), ("q", 0, 3), ("k", 1, 0), ("k", 1, 1), ("k", 1, 2),
                ("k", 1, 3), ("v", 1, 0), ("v", 1, 1), ("v", 1, 2), ("v", 1, 3),
                ("q", 1, 0), ("q", 1, 1), ("q", 1, 2), ("q", 1, 3),
            ]
            for item in dma_order:
                if item == "WO":
                    nc.sync.dma_start(wo_t[:], wo[:, :])
                else:
                    emit_xdma(*item)

            # ---- building blocks ----
            proj_ps = {}

            def proj_mm_half(p, b, hf, dest, part):
                """half (4 matmuls) of an 8-mm accumulation over 512 tokens."""
                key = (p, b, hf)
                if part == 0:
                    proj_ps[key] = psM.tile([128, 512], F32, name="psproj", tag="m")
                ps = proj_ps[key]
                xt = xt_tiles[key]
                for dt in range(4 * part, 4 * part + 4):
                    nc.tensor.matmul(ps[:], w_t[p][:, dt, :], xt[:, dt, :],
                                     start=(dt == 0), stop=(dt == DT - 1))
                if part == 1:
                    del xt_tiles[key], proj_ps[key]
                    nc.vector.tensor_copy(dest, ps[:])

            vstore = {}

            def v_proj_half(b, hf, part):
                if part == 0:
                    vstore[(b, hf)] = vtpool.tile([128, 512], BF16, name="vt", tag="vt")
                proj_mm_half("v", b, hf, vstore[(b, hf)][:], part)

            def v_tp(b, hf, jj):
                """transpose 2 of the 4 key-tiles of v half hf into v_b."""
                vt = vstore[(b, hf)]
                for u in range(2):
                    q = 2 * jj + u
                    jt = 4 * hf + q
                    tp = psM.tile([128, 128], BF16, name="tp", tag="m")
                    nc.tensor.transpose(
                        tp[:], vt[:, q * 128:(q + 1) * 128], ident[:])
                    nc.vector.tensor_copy(v_b[b][:, jt, 0:64], tp[:, 0:64])
                    nc.vector.tensor_copy(v_b[b][:, jt, 65:129], tp[:, 64:128])

            sj_t = {}
            ej_t = {}

            def emit_scores(g):
                c, j = divmod(g, JT)
                b, qc = divmod(c, 4)
                sj = psS.tile([128, 2, QC], F32, name="sj", tag="sj")
                for h in range(2):
                    nc.tensor.matmul(
                        sj[:, h, :], kT[b][h * 64:(h + 1) * 64, j * 128:(j + 1) * 128],
                        qT[b][h * 64:(h + 1) * 64, qc * QC:(qc + 1) * QC],
                        start=True, stop=True)
                sj_t[g] = sj

            def emit_exp(g):
                ej = epool.tile([128, 2, QC], EJ_DT, name="ej", tag="ej")
                nc.scalar.activation(ej[:], sj_t.pop(g)[:], AFT.Exp)
                ej_t[g] = ej

            def emit_av(g, pacc):
                c, j = divmod(g, JT)
                b = c // 4
                ej = ej_t.pop(g)
                for h in range(2):
                    nc.tensor.matmul(
                        pacc[h][0:65, :], v_b[b][:, j, h * 65:h * 65 + 65],
                        ej[:, h, :], start=(j == 0), stop=(j == JT - 1))

            def emit_normalize_h(c, pacc, h):
                dsb = npool.tile([1, QC], F32, name="dsb", tag="dsb")
                nc.vector.tensor_copy(dsb[:], pacc[h][64:65, :])
                rd = npool.tile([1, QC], F32, name="rd", tag="rd")
                nc.vector.reciprocal_approx_fast(out=rd[:], in_=dsb[:])
                bc = npool.tile([64, QC], F32, name="bc", tag="bc")
                nc.gpsimd.partition_broadcast(bc[:], rd[:])
                nc.vector.tensor_tensor(
                    out=xT[h * 64:(h + 1) * 64, c * QC:(c + 1) * QC],
                    in0=pacc[h][0:64, :], in1=bc[:], op=MUL)

            def emit_oproj(tt, use_act=False):
                osb = opool.tile([128, D], OUT_DT, name="osb", tag="osb")
                for ec in range(2):
                    po = psM.tile([128, 512], F32, name="po", tag="m")
                    nc.tensor.matmul(po[:], xT[:, tt * 128:(tt + 1) * 128],
                                     wo_t[:, ec * 512:(ec + 1) * 512],
                                     start=True, stop=True)
                    if use_act and ec == 0:
                        nc.scalar.copy(osb[:, 0:512], po[:])
                    else:
                        nc.vector.tensor_copy(osb[:, ec * 512:(ec + 1) * 512], po[:])
                # last two chunks' outputs ride the (empty by then) sync
                # HWDGE ring at full rate; earlier tiles trickle on SWDGE.
                eng = nc.sync if tt >= 24 else nc.gpsimd
                eng.dma_start(out[tt * 128:(tt + 1) * 128, :], osb[:])

            # ---- fold schedule (each item <=~1us of PE) ----
            def k_half(hf, part):
                return lambda: proj_mm_half(
                    "k", 0, hf, kT[0][:, hf * 512:(hf + 1) * 512], part)

            def k1_half(hf, part):
                return lambda: proj_mm_half(
                    "k", 1, hf, kT[1][:, hf * 512:(hf + 1) * 512], part)

            def q_half(c, part):
                b, qc = divmod(c, 4)
                return lambda: proj_mm_half(
                    "q", b, qc, qT[b][:, qc * 512:(qc + 1) * 512], part)

            def vp_half(b, hf, part):
                return lambda: v_proj_half(b, hf, part)

            def vt_item(b, hf, jj):
                return lambda: v_tp(b, hf, jj)

            def op_item(tt):
                return lambda: emit_oproj(tt)

            FOLD = {}

            def put(g, th):
                FOLD.setdefault(g, []).append(th)

            def put_pair(g0, mk, *args):
                put(g0, mk(*args, 0))
                put(g0 + 1, mk(*args, 1))

            # b0 projections + v prep, arrival-matched to the DMA stream
            put_pair(0, k_half, 1)
            put_pair(2, vp_half, 0, 0)
            put_pair(4, k_half, 2)
            put(6, vt_item(0, 0, 0)); put(6, vt_item(0, 0, 1))
            put_pair(7, vp_half, 0, 1)
            put_pair(8, k_half, 3)
            put(11, vt_item(0, 1, 0)); put(11, vt_item(0, 1, 1))
            put_pair(12, q_half, 1)
            put_pair(14, vp_half, 0, 2)
            put(16, vt_item(0, 2, 0)); put(16, vt_item(0, 2, 1))
            put_pair(17, vp_half, 0, 3)
            put(19, vt_item(0, 3, 0)); put(19, vt_item(0, 3, 1))
            put_pair(26, q_half, 2)
            # b1 prep under b0's chunks 2-3; q03 first — its deadline
            # (scores(48) emitted at slot 46) has no slack, k1x's does.
            put_pair(34, q_half, 3)
            put_pair(36, k1_half, 0)
            put_pair(38, k1_half, 1)
            put_pair(40, k1_half, 2)
            put_pair(42, k1_half, 3)
            put_pair(46, vp_half, 1, 0)
            put(48, vt_item(1, 0, 0)); put(48, vt_item(1, 0, 1))
            put_pair(49, vp_half, 1, 1)
            put(51, vt_item(1, 1, 0)); put(51, vt_item(1, 1, 1))
            put_pair(52, vp_half, 1, 2)
            put(54, vt_item(1, 2, 0)); put(54, vt_item(1, 2, 1))
            put_pair(55, vp_half, 1, 3)
            put(57, vt_item(1, 3, 0)); put(57, vt_item(1, 3, 1))
            put_pair(58, q_half, 4)
            put_pair(74, q_half, 5)
            put_pair(90, q_half, 6)
            put_pair(106, q_half, 7)
            # output projection tiles, spread into fold-free slots
            op_slots = {0: (29, 31, 33, 44), 1: (64, 66, 68, 70),
                        2: (72, 76, 78, 80), 3: (82, 84, 86, 88),
                        4: (92, 94, 96, 98), 5: (100, 102, 104, 108),
                        6: (114, 116, 118, 120)}
            for c, slots in op_slots.items():
                for i, g in enumerate(slots):
                    put(g, op_item(4 * c + i))

            # per-chunk av lag: chunk 0 rides out the input DMA ramp
            AV_LAG = {c: (11 if c == 0 else 1) for c in range(NCH)}
            AV_AT = {}
            for g in range(NSLOT):
                AV_AT.setdefault(g + AV_LAG[g // JT], []).append(g)

            # ---- prologue: first k/q projections ----
            for part in (0, 1):
                proj_mm_half("k", 0, 0, kT[0][:, 0:512], part)
            for part in (0, 1):
                proj_mm_half("q", 0, 0, qT[0][:, 0:512], part)
            emit_scores(0)
            emit_scores(1)

            # ---- the stream ----
            paccs = {}

            norm_pend = {}

            def run_avs(slot):
                # second normalize half from the previous chunk, one slot
                # later, so the vector burst at a chunk boundary is halved
                for ca, pacc in list(norm_pend.items()):
                    emit_normalize_h(ca, pacc, 1)
                    del norm_pend[ca]
                for ga in AV_AT.get(slot, ()):
                    ca = ga // JT
                    if ga % JT == 0:
                        paccs[ca] = [psA.tile([128, QC], F32, name=f"pacc{h}",
                                              tag="pacc") for h in range(2)]
                    emit_av(ga, paccs[ca])
                    if ga % JT == JT - 1:
                        pacc = paccs.pop(ca)
                        emit_normalize_h(ca, pacc, 0)
                        norm_pend[ca] = pacc

            for g in range(NSLOT):
                emit_exp(g)
                if g + 2 < NSLOT:
                    emit_scores(g + 2)
                for th in FOLD.get(g, ()):
                    th()
                run_avs(g)

            # ---- tail: trailing avs, then last chunk's output projection ----
            for slot in range(NSLOT, NSLOT + max(AV_LAG.values()) + 1):
                run_avs(slot)
            for tt in range(28, 32):
                emit_oproj(tt, use_act=True)
    nc.finalize()
    return nc


@functools.cache
def _nc_cached():
    return build_nc()


def _prep_in_maps(inputs):
    np16 = np.float16

    def xbm(a):
        # [S,B,D] -> [128, (b hf) a m]: tile-contiguous per partition
        xd = np.asarray(a, np.float32).transpose(2, 1, 0)        # [D, B, S]
        xd = xd.reshape(DT, 128, B, 4, 512).transpose(1, 2, 3, 0, 4)
        return np.ascontiguousarray(xd.reshape(128, B * 4 * DT * 512)).astype(np16)

    xq_h = xbm(inputs["query"])
    xk_h = xbm(inputs["key"])
    xv_h = xbm(inputs["value"])
    Wq, Wk, Wv, Wo = (np.asarray(inputs[k], np.float32) for k in ("Wq", "Wk", "Wv", "Wo"))

    def wpack(w):
        # [D, FPC] (t p)-major -> [p=128, t*m] contiguous per partition
        return np.ascontiguousarray(
            w.reshape(DT, 128, FPC).transpose(1, 0, 2).reshape(128, DT * FPC)
        ).astype(np16)

    in_maps = []
    for c in range(NC):
        sl = slice(c * FPC, (c + 1) * FPC)
        in_maps.append({
            "xq": xq_h, "xk": xk_h, "xv": xv_h,
            "wq": wpack(Wq[sl, :].T),
            "wk": wpack(Wk[sl, :].T),
            "wv": wpack(Wv[sl, :].T),
            "wo": np.ascontiguousarray(Wo[:, sl].T).astype(np16),
        })
    return in_maps


def kernel(query, key, value, Wq, bq, Wk, bk, Wv, bv, Wo, bo):
    in_maps = _prep_in_maps({"query": query, "key": key, "value": value,
                             "Wq": Wq, "Wk": Wk, "Wv": Wv, "Wo": Wo})
    nc = _nc_cached()
    res = run_bass_kernel_spmd(nc, in_maps, core_ids=list(range(NC)))
    acc = np.zeros((T, D), np.float32)
    for r in res.results:
        acc += r["out"].astype(np.float32)
    acc += np.asarray(bo, np.float32)[None, :]
    # batch-major tokens back to [S, B, D]
    out = acc.reshape(B, S, D).transpose(1, 0, 2)
    out = np.ascontiguousarray(out, np.float32)
    for bias in (bq, bk, bv):
        assert float(np.abs(np.asarray(bias)).max()) == 0.0, "nonzero qkv bias unsupported"
    return out


# revision 29
# speedup vs baseline: 1.1228x; 1.0130x over previous
"""Multi-head attention (S=2048, B=2, D=1024, H=16) on 8 trn2 NeuronCores.

Sharding: 2 heads per core (head parallelism). Each core computes Q/K/V
projections for its 128 output features, attention for its 4 (batch,
head) pairs, and a partial output projection; the host sums the 8
partial outputs.

v4 restructure (from the 242us v3 trace): the TENSOR engine is the
bottleneck (~150us effective queue time), not the exp stream (~134us
busy), so the schedule keeps the tensor FIFO never-idle:
 - av matmuls and v_b/ej are bf16 (f32r moving operands streamed at
   half rate: 422ns -> 216ns per av matmul).
 - every 8-matmul projection group is split into 4+4 halves folded at
   adjacent slots so no fold bursts >1us delay the next scores pair
   (which gates exp via the 2-slot sj lead).
 - av(g) runs at slot g+1 (chunk 0: g+11, riding out the input DMA
   ramp) so an av never head-blocks the FIFO waiting on its exp.
 - consumption-ordered input DMA: k/q first-chunk halves, then k01..
   k03/v00..v03/q01 interleaved to match per-slot needs; weights ride
   the scalar HWDGE ring so the x stream owns the sync ring at t=0.
 - exp activation table preloaded at t~0 (dummy exp) to shave the
   2.7us ACT_TABLE_LOAD off the first real exp.
 - outputs: last two chunks on the (empty by then) sync HWDGE ring;
   earlier tiles trickle on the gpsimd SWDGE ring.
"""
import sys
sys.path.insert(0, '/opt/trn_rl_repo')
import functools
import os

import numpy as np

import concourse.bacc as bacc
import concourse.mybir as mybir
import concourse.tile as tile
from concourse.bass_utils import run_bass_kernel_spmd
from concourse.masks import make_identity

F32 = mybir.dt.float32
F32R = mybir.dt.float32r
F16 = mybir.dt.float16
BF16 = mybir.dt.bfloat16
AFT = mybir.ActivationFunctionType
MUL = mybir.AluOpType.mult

S, B, D, H = 2048, 2, 1024, 16
T = S * B               # 4096 tokens
DK = D // H             # 64
NC = 8                  # cores
FPC = D // NC           # 128 features per core (2 heads)
QC = 512                # q-chunk size
JT = S // 128           # 16 key tiles per batch
DT = D // 128           # 8 contraction tiles for projections
NCH = 8                 # chunks, b-outer: c -> b=c//4, qc=c%4
NSLOT = NCH * JT        # 128 global (chunk, key-tile) slots

EJ_DT = {"f32r": F32R, "bf16": BF16}[os.environ.get("EJ_DT", "bf16")]
OUT_DT = {"fp16": F16, "fp32": F32}[os.environ.get("OUT_DT", "fp16")]


def build_nc():
    nc = bacc.Bacc(None, target_bir_lowering=False)

    # x pre-arranged host-side as [128, (b hf) a m]: every 512-token tile is
    # contiguous per partition -> 128 DMA descriptors per tile, not 1024.
    XW = B * 4 * DT * 512
    xq = nc.dram_tensor("xq", [128, XW], F16, kind="ExternalInput")
    xk = nc.dram_tensor("xk", [128, XW], F16, kind="ExternalInput")
    xv = nc.dram_tensor("xv", [128, XW], F16, kind="ExternalInput")
    wq = nc.dram_tensor("wq", [128, DT * FPC], F16, kind="ExternalInput")
    wk = nc.dram_tensor("wk", [128, DT * FPC], F16, kind="ExternalInput")
    wv = nc.dram_tensor("wv", [128, DT * FPC], F16, kind="ExternalInput")
    wo = nc.dram_tensor("wo", [FPC, D], F16, kind="ExternalInput")
    out = nc.dram_tensor("out", [T, D], OUT_DT, kind="ExternalOutput")
    xsrc = {"q": xq, "k": xk, "v": xv}

    with tile.TileContext(nc) as tc:
        with (
            tc.tile_pool(name="wpool", bufs=1) as wpool,
            tc.tile_pool(name="proj", bufs=1) as projpool,
            tc.tile_pool(name="vtmp", bufs=2) as vtpool,
            tc.tile_pool(name="xdma", bufs=8) as xpool,
            tc.tile_pool(name="ej", bufs=26) as epool,
            tc.tile_pool(name="norm", bufs=2) as npool,
            tc.tile_pool(name="osb", bufs=3) as opool,
            tc.tile_pool(name="psS", bufs=2, space="PSUM") as psS,
            tc.tile_pool(name="psA", bufs=2, space="PSUM") as psA,
            tc.tile_pool(name="psM", bufs=2, space="PSUM") as psM,
        ):
            # ---- exp table preload: dummy exp at t~0 so the first real
            # exp doesn't pay the ~2.7us ACT_TABLE_LOAD.
            warm = npool.tile([1, 8], F32, name="warm", tag="warm")
            warm_o = npool.tile([1, 8], F32, name="warm_o", tag="warm_o")
            nc.vector.memset(warm[:], 0.0)
            nc.scalar.activation(warm_o[:], warm[:], AFT.Exp)

            # ---- weights / constants ----
            w_t = {}
            for name, wd in (("k", wk), ("q", wq), ("v", wv)):
                w_t[name] = wpool.tile([128, DT, FPC], F16, name=f"w_{name}")
                # pre-packed [128, DT*FPC] host-side: contiguous per
                # partition; scalar HWDGE ring so the x stream owns sync.
                nc.scalar.dma_start(w_t[name][:], wd.rearrange("p (t m) -> p t m", t=DT))
            ident = wpool.tile([128, 128], BF16, name="ident")
            make_identity(nc, ident[:])
            wo_t = wpool.tile([128, D], F16, name="wo_t")

            # ---- persistent activations ----
            kT = [projpool.tile([128, S], F16, name=f"kT{b}") for b in range(B)]
            qT = [projpool.tile([128, S], F16, name=f"qT{b}") for b in range(B)]
            v_b = [projpool.tile([128, JT, 130], BF16, name=f"v_b{b}") for b in range(B)]
            xT = projpool.tile([128, T], F16, name="xT")
            # ones columns of v_b (denominator trick) are static
            for b in range(B):
                for jt in range(JT):
                    nc.vector.memset(v_b[b][:, jt, 64:65], 1.0)
                    nc.vector.memset(v_b[b][:, jt, 129:130], 1.0)

            # ---- input DMA stream (emission order == transfer order) ----
            xt_tiles = {}
            xh_tiles = {}

            def emit_xdma(p, b, hf):
                t = xpool.tile([128, DT, 512], F16, name="xt", tag="xt")
                idx = b * 4 + hf
                nc.sync.dma_start(
                    t[:],
                    xsrc[p][:, idx * DT * 512:(idx + 1) * DT * 512]
                    .rearrange("p (a m) -> p a m", a=DT))
                xt_tiles[(p, b, hf)] = t

            emit_xdma("k", 0, 0)
            emit_xdma("q", 0, 0)
            dma_order = [
                ("k", 0, 1), ("v", 0, 0), ("k", 0, 2), ("v", 0, 1), ("k", 0, 3),
                ("v", 0, 2), ("q", 0, 1), ("v", 0, 3),
                "WO",
                ("q", 0, 2), ("q", 0, 3), ("k", 1, 0), ("k", 1, 1), ("k", 1, 2),
                ("k", 1, 3), ("v", 1, 0), ("v", 1, 1), ("v", 1, 2), ("v", 1, 3),
                ("q", 1, 0), ("q", 1, 1), ("q", 1, 2), ("q", 1, 3),
            ]
            for item in dma_order:
                if item == "WO":
                    nc.sync.dma_start(wo_t[:], wo[:, :])
                else:
                    emit_xdma(*item)

            # ---- building blocks ----
            proj_ps = {}

            def proj_mm_half(p, b, hf, dest, part):
                """half (4 matmuls) of an 8-mm accumulation over 512 tokens."""
                key = (p, b, hf)
                if part == 0:
                    proj_ps[key] = psM.tile([128, 512], F32, name="psproj", tag="m")
                ps = proj_ps[key]
                xt = xt_tiles[key]
                for dt in range(4 * part, 4 * part + 4):
                    nc.tensor.matmul(ps[:], w_t[p][:, dt, :], xt[:, dt, :],
                                     start=(dt == 0), stop=(dt == DT - 1))
                if part == 1:
                    del xt_tiles[key], proj_ps[key]
                    nc.vector.tensor_copy(dest, ps[:])

            vstore = {}

            def v_proj_half(b, hf, part):
                if part == 0:
                    vstore[(b, hf)] = vtpool.tile([128, 512], BF16, name="vt", tag="vt")
                proj_mm_half("v", b, hf, vstore[(b, hf)][:], part)

            def v_tp(b, hf, jj):
                """transpose 2 of the 4 key-tiles of v half hf into v_b."""
                vt = vstore[(b, hf)]
                for u in range(2):
                    q = 2 * jj + u
                    jt = 4 * hf + q
                    tp = psM.tile([128, 128], BF16, name="tp", tag="m")
                    nc.tensor.transpose(
                        tp[:], vt[:, q * 128:(q + 1) * 128], ident[:])
                    nc.vector.tensor_copy(v_b[b][:, jt, 0:64], tp[:, 0:64])
                    nc.vector.tensor_copy(v_b[b][:, jt, 65:129], tp[:, 64:128])

            sj_t = {}
            ej_t = {}

            def emit_scores(g):
                c, j = divmod(g, JT)
                b, qc = divmod(c, 4)
                sj = psS.tile([128, 2, QC], F32, name="sj", tag="sj")
                for h in range(2):
                    nc.tensor.matmul(
                        sj[:, h, :], kT[b][h * 64:(h + 1) * 64, j * 128:(j + 1) * 128],
                        qT[b][h * 64:(h + 1) * 64, qc * QC:(qc + 1) * QC],
                        start=True, stop=True)
                sj_t[g] = sj

            def emit_exp(g):
                ej = epool.tile([128, 2, QC], EJ_DT, name="ej", tag="ej")
                nc.scalar.activation(ej[:], sj_t.pop(g)[:], AFT.Exp)
                ej_t[g] = ej

            def emit_av(g, pacc):
                c, j = divmod(g, JT)
                b = c // 4
                ej = ej_t.pop(g)
                for h in range(2):
                    nc.tensor.matmul(
                        pacc[h][0:65, :], v_b[b][:, j, h * 65:h * 65 + 65],
                        ej[:, h, :], start=(j == 0), stop=(j == JT - 1))

            def emit_normalize_h(c, pacc, h):
                dsb = npool.tile([1, QC], F32, name="dsb", tag="dsb")
                # for the last chunk the scalar engine is idle (exp done) —
                # use it for the PSUM read so h0/h1 chains overlap
                if c == NCH - 1:
                    nc.scalar.copy(dsb[:], pacc[h][64:65, :])
                else:
                    nc.vector.tensor_copy(dsb[:], pacc[h][64:65, :])
                rd = npool.tile([1, QC], F32, name="rd", tag="rd")
                nc.vector.reciprocal_approx_fast(out=rd[:], in_=dsb[:])
                bc = npool.tile([64, QC], F32, name="bc", tag="bc")
                nc.gpsimd.partition_broadcast(bc[:], rd[:])
                nc.vector.tensor_tensor(
                    out=xT[h * 64:(h + 1) * 64, c * QC:(c + 1) * QC],
                    in0=pacc[h][0:64, :], in1=bc[:], op=MUL)

            def emit_oproj(tt, use_act=False):
                osb = opool.tile([128, D], OUT_DT, name="osb", tag="osb")
                for ec in range(2):
                    po = psM.tile([128, 512], F32, name="po", tag="m")
                    nc.tensor.matmul(po[:], xT[:, tt * 128:(tt + 1) * 128],
                                     wo_t[:, ec * 512:(ec + 1) * 512],
                                     start=True, stop=True)
                    if use_act and ec == 0:
                        nc.scalar.copy(osb[:, 0:512], po[:])
                    else:
                        nc.vector.tensor_copy(osb[:, ec * 512:(ec + 1) * 512], po[:])
                # last two chunks' outputs ride the (empty by then) sync
                # HWDGE ring at full rate; earlier tiles trickle on SWDGE.
                eng = nc.sync if tt >= 24 else nc.gpsimd
                eng.dma_start(out[tt * 128:(tt + 1) * 128, :], osb[:])

            # ---- fold schedule (each item <=~1us of PE) ----
            def k_half(hf, part):
                return lambda: proj_mm_half(
                    "k", 0, hf, kT[0][:, hf * 512:(hf + 1) * 512], part)

            def k1_half(hf, part):
                return lambda: proj_mm_half(
                    "k", 1, hf, kT[1][:, hf * 512:(hf + 1) * 512], part)

            def q_half(c, part):
                b, qc = divmod(c, 4)
                return lambda: proj_mm_half(
                    "q", b, qc, qT[b][:, qc * 512:(qc + 1) * 512], part)

            def vp_half(b, hf, part):
                return lambda: v_proj_half(b, hf, part)

            def vt_item(b, hf, jj):
                return lambda: v_tp(b, hf, jj)

            def op_item(tt):
                return lambda: emit_oproj(tt)

            FOLD = {}

            def put(g, th):
                FOLD.setdefault(g, []).append(th)

            def put_pair(g0, mk, *args):
                put(g0, mk(*args, 0))
                put(g0 + 1, mk(*args, 1))

            # b0 projections + v prep, arrival-matched to the DMA stream
            put_pair(0, k_half, 1)
            put_pair(2, vp_half, 0, 0)
            put_pair(4, k_half, 2)
            put(6, vt_item(0, 0, 0)); put(6, vt_item(0, 0, 1))
            put_pair(7, vp_half, 0, 1)
            put_pair(8, k_half, 3)
            put(11, vt_item(0, 1, 0)); put(11, vt_item(0, 1, 1))
            put_pair(12, q_half, 1)
            put_pair(14, vp_half, 0, 2)
            put(16, vt_item(0, 2, 0)); put(16, vt_item(0, 2, 1))
            put_pair(17, vp_half, 0, 3)
            put(19, vt_item(0, 3, 0)); put(19, vt_item(0, 3, 1))
            put_pair(26, q_half, 2)
            # b1 prep under b0's chunks 2-3; q03 first — its deadline
            # (scores(48) emitted at slot 46) has no slack, k1x's does.
            put_pair(34, q_half, 3)
            put_pair(44, k1_half, 0)
            put_pair(46, k1_half, 1)
            put_pair(48, k1_half, 2)
            put_pair(50, k1_half, 3)
            put_pair(52, q_half, 4)
            put_pair(54, vp_half, 1, 0)
            put(56, vt_item(1, 0, 0)); put(57, vt_item(1, 0, 1))
            put_pair(58, vp_half, 1, 1)
            put(60, vt_item(1, 1, 0)); put(61, vt_item(1, 1, 1))
            put_pair(62, vp_half, 1, 2)
            put(64, vt_item(1, 2, 0)); put(65, vt_item(1, 2, 1))
            put_pair(66, vp_half, 1, 3)
            put(68, vt_item(1, 3, 0)); put(69, vt_item(1, 3, 1))
            put_pair(74, q_half, 5)
            put_pair(90, q_half, 6)
            put_pair(106, q_half, 7)
            # output projection tiles, spread into fold-free slots
            op_slots = {0: (29, 31, 33, 42), 1: (70, 72, 76, 78),
                        2: (80, 82, 84, 86), 3: (88, 92, 94, 96),
                        4: (98, 100, 102, 104), 5: (108, 110, 112, 114),
                        6: (116, 118, 120, 121)}
            for c, slots in op_slots.items():
                for i, g in enumerate(slots):
                    put(g, op_item(4 * c + i))

            # per-chunk av lag: chunk 0 rides out the input DMA ramp
            AV_LAG = {c: (11 if c == 0 else 1) for c in range(NCH)}
            AV_AT = {}
            for g in range(NSLOT):
                AV_AT.setdefault(g + AV_LAG[g // JT], []).append(g)

            # ---- prologue: first k/q projections ----
            for part in (0, 1):
                proj_mm_half("k", 0, 0, kT[0][:, 0:512], part)
            for part in (0, 1):
                proj_mm_half("q", 0, 0, qT[0][:, 0:512], part)
            emit_scores(0)
            emit_scores(1)

            # ---- the stream ----
            paccs = {}

            norm_pend = {}

            def run_avs(slot):
                # second normalize half from the previous chunk, one slot
                # later, so the vector burst at a chunk boundary is halved
                for ca, pacc in list(norm_pend.items()):
                    emit_normalize_h(ca, pacc, 1)
                    del norm_pend[ca]
                for ga in AV_AT.get(slot, ()):
                    ca = ga // JT
                    if ga % JT == 0:
                        paccs[ca] = [psA.tile([128, QC], F32, name=f"pacc{h}",
                                              tag="pacc") for h in range(2)]
                    emit_av(ga, paccs[ca])
                    if ga % JT == JT - 1:
                        pacc = paccs.pop(ca)
                        emit_normalize_h(ca, pacc, 0)
                        norm_pend[ca] = pacc

            for g in range(NSLOT):
                emit_exp(g)
                if g + 2 < NSLOT:
                    emit_scores(g + 2)
                for th in FOLD.get(g, ()):
                    th()
                run_avs(g)

            # ---- tail: trailing avs, then last chunk's output projection ----
            for slot in range(NSLOT, NSLOT + max(AV_LAG.values()) + 1):
                run_avs(slot)
            for tt in range(28, 32):
                emit_oproj(tt, use_act=True)
    nc.finalize()
    return nc


@functools.cache
def _nc_cached():
    return build_nc()


def _prep_in_maps(inputs):
    np16 = np.float16

    def xbm(a):
        # [S,B,D] -> [128, (b hf) a m]: tile-contiguous per partition
        xd = np.asarray(a, np.float32).transpose(2, 1, 0)        # [D, B, S]
        xd = xd.reshape(DT, 128, B, 4, 512).transpose(1, 2, 3, 0, 4)
        return np.ascontiguousarray(xd.reshape(128, B * 4 * DT * 512)).astype(np16)

    xq_h = xbm(inputs["query"])
    xk_h = xbm(inputs["key"])
    xv_h = xbm(inputs["value"])
    Wq, Wk, Wv, Wo = (np.asarray(inputs[k], np.float32) for k in ("Wq", "Wk", "Wv", "Wo"))

    def wpack(w):
        # [D, FPC] (t p)-major -> [p=128, t*m] contiguous per partition
        return np.ascontiguousarray(
            w.reshape(DT, 128, FPC).transpose(1, 0, 2).reshape(128, DT * FPC)
        ).astype(np16)

    in_maps = []
    for c in range(NC):
        sl = slice(c * FPC, (c + 1) * FPC)
        in_maps.append({
            "xq": xq_h, "xk": xk_h, "xv": xv_h,
            "wq": wpack(Wq[sl, :].T),
            "wk": wpack(Wk[sl, :].T),
            "wv": wpack(Wv[sl, :].T),
            "wo": np.ascontiguousarray(Wo[:, sl].T).astype(np16),
        })
    return in_maps


def kernel(query, key, value, Wq, bq, Wk, bk, Wv, bv, Wo, bo):
    in_maps = _prep_in_maps({"query": query, "key": key, "value": value,
                             "Wq": Wq, "Wk": Wk, "Wv": Wv, "Wo": Wo})
    nc = _nc_cached()
    res = run_bass_kernel_spmd(nc, in_maps, core_ids=list(range(NC)))
    acc = np.zeros((T, D), np.float32)
    for r in res.results:
        acc += r["out"].astype(np.float32)
    acc += np.asarray(bo, np.float32)[None, :]
    # batch-major tokens back to [S, B, D]
    out = acc.reshape(B, S, D).transpose(1, 0, 2)
    out = np.ascontiguousarray(out, np.float32)
    for bias in (bq, bk, bv):
        assert float(np.abs(np.asarray(bias)).max()) == 0.0, "nonzero qkv bias unsupported"
    return out
